# revision 1
# baseline (speedup 1.0000x reference)
"""Multi-head attention (B=4, S=2048, H=16, d_model=1024, d_k=d_v=64) on 8
Trainium2 NeuronCores.

Sharding: 8 cores = 4 batches x 2 query-halves. Each core computes all 16
heads for its (batch, query-half); K/V projections are recomputed per
query-half so no inter-core communication is needed; outputs are disjoint
and concatenated on the host.

Host prep: Q/K/V are transposed to [d_model, seq] (V additionally blocked
by s-chunk) and cast to bf16 on the host, as are all weights, so the kernel
needs no on-chip transposes of the activations.

Per-core pipeline:
  - projections: kt[p] = W_K[pair p].T @ K -> SBUF [128, S] bf16 per pair;
    qt[p] likewise [128, QH]; v_all = per s-chunk blocks of 16 head-slots
    [1|v] (65 wide, bf16)
  - attention per pair, software-pipelined: only pair 0's K/Q projections
    run up front; all V projection groups (quarter-width, N=256) and the
    remaining K/Q pairs are emitted as PE filler work inside the attention
    loop (which is otherwise paced by the ACT engine's exp).
    scoresT = kt-chunk.T @ qt (PSUM f32), e = exp(s/8) on ACT -> bf16, then
    the flipped value matmul o[q, 65] = e-chunk.T @ [1|v] (moving operand
    only 65 columns) accumulated over s-chunks; column 0 is the softmax
    denominator.
  - normalize: one broadcast multiply by 1/denom into the pair's dead kt
    tile, then PE re-transpose (x identity) into the pair's qt tile as
    pair-stacked headsT [128, QH] (scheduled as filler in pairs 6-7).
  - output projection: out = concat(heads) @ W_O accumulated over 8
    pair-chunks.

PSUM note: start_tensor_calc marks the whole 2KB bank pending-zero, so
interleaved per-slot accumulation groups sharing a bank must issue exactly
one start (first slot); the other slots' first writes land on pending-zero
bytes, which the hardware treats as overwrite.
"""

import contextlib
import os
import sys

for _p in ("/opt/trn_rl_repo", "/root/.axon_site/_ro/trn_rl_repo"):
    if os.path.isdir(_p) and _p not in sys.path:
        sys.path.insert(0, _p)

import numpy as np
import ml_dtypes

import concourse.bass as bass  # noqa: F401
import concourse.tile as tile
from concourse import bacc, mybir
from concourse.bass_utils import run_bass_kernel_spmd
from concourse.masks import make_identity

F32 = mybir.dt.float32
BF16 = mybir.dt.bfloat16

B, S, DM = 4, 2048, 1024
H, D = 16, 64
QH = S // 2  # query half per core
N_CORES = 8
NP = H // 2  # head pairs
N_SC = S // 128  # kv 128-chunks
N_MO = DM // 128  # model-dim 128-chunks
N_QC = QH // 128  # q 128-chunks


def build(n_cores=N_CORES, phases=(1, 2, 3), dbg=False):
    nc = bacc.Bacc("TRN2", target_bir_lowering=False, debug=False, num_devices=n_cores)

    # host-transposed activations, bf16
    qt_d = nc.dram_tensor("QT", [N_MO, 128, QH], BF16, kind="ExternalInput").ap()
    kt_d = nc.dram_tensor("KT", [N_MO, 128, S], BF16, kind="ExternalInput").ap()
    # V blocked by s-chunk: [sc, 128(dm within mo), mo, 128(s within chunk)]
    vt_d = nc.dram_tensor(
        "VTs", [N_SC, 128, N_MO, 128], BF16, kind="ExternalInput"
    ).ap()
    # host-prepped weights, bf16; K/Q pair-major: [pair, mi=128, mo=8, 128]
    w_q = nc.dram_tensor(
        "WQP", [NP, 128, N_MO, 128], BF16, kind="ExternalInput"
    ).ap()
    w_k = nc.dram_tensor(
        "WKP", [NP, 128, N_MO, 128], BF16, kind="ExternalInput"
    ).ap()
    w_v = nc.dram_tensor("WV3", [128, N_MO, H * D], BF16, kind="ExternalInput").ap()
    # [mi=128, pair-chunk=8, dm=1024]
    w_o = nc.dram_tensor("WO3", [128, NP, DM], BF16, kind="ExternalInput").ap()
    out = nc.dram_tensor("out", [QH, DM], F32, kind="ExternalOutput").ap()
    if dbg:
        d_kt = nc.dram_tensor("d_kt", [128, S], BF16, kind="ExternalOutput").ap()
        d_qt = nc.dram_tensor("d_qt", [128, QH], BF16, kind="ExternalOutput").ap()
        d_vall = nc.dram_tensor(
            "d_vall", [128, N_SC * H * 65], BF16, kind="ExternalOutput"
        ).ap()
        d_onorm = nc.dram_tensor(
            "d_onorm", [128, QH], BF16, kind="ExternalOutput"
        ).ap()
        d_heads = nc.dram_tensor(
            "d_heads", [128, QH], BF16, kind="ExternalOutput"
        ).ap()

    with tile.TileContext(nc) as tc:
        with (
            tc.tile_pool(name="pers", bufs=1) as pers,
            tc.tile_pool(name="wkq", bufs=2) as wkq,
        ):
            ident_f32 = pers.tile([128, 128], F32)
            make_identity(nc, ident_f32[:])
            ident = pers.tile([128, 128], BF16)
            nc.vector.tensor_copy(ident[:], ident_f32[:])

            # v resident: per s-chunk block of 16 head-slots [1|v] (65 wide)
            v_all = pers.tile([128, N_SC, H, 65], BF16, tag="v_all")
            nc.vector.memset(v_all[:, :, :, 0:1], 1.0)
            # kt[p]: pair-stacked [2*64, S]; qt[p]: [2*64, QH] -> later
            # reused for normalized flipped heads, then pair-stacked headsT.
            kt_sb = [
                pers.tile([128, S], BF16, tag=f"kt{p}", name=f"kt{p}")
                for p in range(NP)
            ]
            qt_sb = [
                pers.tile([128, QH], BF16, tag=f"qt{p}", name=f"qt{p}")
                for p in range(NP)
            ]
            # flipped normalized heads, aliasing kt (dead after pair's scores)
            o_norm = [
                kt_sb[p][:, 0:1024].rearrange("p (s w) -> p s w", s=16)
                for p in range(NP)
            ]
            # resident staged transposed inputs for projections
            ktx = pers.tile([128, N_MO, S], BF16, tag="ktx")
            qtx = pers.tile([128, N_MO, QH], BF16, tag="qtx")

            with (
                tc.tile_pool(name="psum_sp", bufs=1, space="PSUM") as spsum,
                tc.tile_pool(name="psum_o", bufs=1, space="PSUM") as opsum,
                tc.tile_pool(name="psum_pj", bufs=1, space="PSUM") as pjsum,
                tc.tile_pool(name="epool", bufs=5) as epool,
                tc.tile_pool(name="npool", bufs=1) as npool,
            ):
                ves = contextlib.ExitStack()
                les = contextlib.ExitStack()
                vpool = ves.enter_context(tc.tile_pool(name="vpool", bufs=1))
                vtx = [
                    vpool.tile([128, N_MO, 128], BF16, tag=f"vt{sc}", name=f"vt{sc}")
                    for sc in range(N_SC)
                ]
                wv_sb = vpool.tile([128, N_MO, H * D], BF16, tag="wv")

                cur_wk, cur_wq = {}, {}

                def prefetch_wk(p):
                    wkt = wkq.tile([128, N_MO, 128], BF16, tag="wk", name="wk")
                    cur_wk[p] = wkt
                    nc.sync.dma_start(out=wkt[:], in_=w_k[p])

                def prefetch_wq(p):
                    wqt = wkq.tile([128, N_MO, 128], BF16, tag="wq", name="wq")
                    cur_wq[p] = wqt
                    nc.sync.dma_start(out=wqt[:], in_=w_q[p])

                def dma_ktx(g):
                    for mo in range(N_MO):
                        nc.sync.dma_start(
                            out=ktx[:, mo, g * 512 : (g + 1) * 512],
                            in_=kt_d[mo, :, g * 512 : (g + 1) * 512],
                        )

                def dma_wv(q):
                    nc.sync.dma_start(
                        out=wv_sb[:, :, q * 256 : (q + 1) * 256],
                        in_=w_v[:, :, q * 256 : (q + 1) * 256],
                    )

                # startup-critical DMA order: pair-0 K/Q weights + first K
                # columns + all of QT, then V / remaining K interleaved
                def dma_vt(sc):
                    nc.sync.dma_start(out=vtx[sc][:], in_=vt_d[sc])

                def dma_qtx(g):
                    for mo in range(N_MO):
                        nc.sync.dma_start(
                            out=qtx[:, mo, g * 512 : (g + 1) * 512],
                            in_=qt_d[mo, :, g * 512 : (g + 1) * 512],
                        )

                prefetch_wk(0)
                prefetch_wq(0)
                dma_ktx(0)
                dma_qtx(0)
                dma_qtx(1)
                dma_wv(0)
                dma_vt(0)
                dma_vt(1)
                dma_ktx(1)
                dma_vt(2)
                dma_ktx(2)
                dma_vt(3)
                dma_ktx(3)
                dma_vt(4)
                prefetch_wk(1)
                prefetch_wq(1)
                dma_vt(5)
                dma_wv(1)
                dma_vt(6)
                dma_wv(2)
                dma_vt(7)
                dma_wv(3)
                for sc in range(8, N_SC):
                    dma_vt(sc)

                # ---------- projection "filler" groups ----------
                def k_group(p, g):
                    pj = pjsum.tile([128, 512], F32, tag="pj")
                    for mo in range(N_MO):
                        nc.tensor.matmul(
                            pj[:],
                            cur_wk[p][:, mo, :],
                            ktx[:, mo, g * 512 : (g + 1) * 512],
                            start=(mo == 0),
                            stop=(mo == N_MO - 1),
                        )
                    nc.vector.tensor_copy(
                        kt_sb[p][:, g * 512 : (g + 1) * 512], pj[:]
                    )

                def q_group(p, g):
                    pj = pjsum.tile([128, 512], F32, tag="pj")
                    for mo in range(N_MO):
                        nc.tensor.matmul(
                            pj[:],
                            cur_wq[p][:, mo, :],
                            qtx[:, mo, g * 512 : (g + 1) * 512],
                            start=(mo == 0),
                            stop=(mo == N_MO - 1),
                        )
                    nc.vector.tensor_copy(
                        qt_sb[p][:, g * 512 : (g + 1) * 512], pj[:]
                    )

                def v_group(sc, quarter):
                    pj = pjsum.tile([128, 512], F32, tag="pj")
                    for mo in range(N_MO):
                        nc.tensor.matmul(
                            pj[:, 0:256],
                            vtx[sc][:, mo, :],
                            wv_sb[:, mo, quarter * 256 : (quarter + 1) * 256],
                            start=(mo == 0),
                            stop=(mo == N_MO - 1),
                        )
                    nc.vector.tensor_copy(
                        v_all[:, sc, quarter * 4 : (quarter + 1) * 4, 1:65],
                        pj[:, 0:256].rearrange("p (h w) -> p h w", h=4),
                    )

                # K/Q for pair 0 before attention starts (columns 0-511 at
                # least; the rest of K0 is the first filler work).
                k_group(0, 0)
                for g in range(2):
                    q_group(0, g)

                if dbg:
                    nc.sync.dma_start(out=d_kt[:], in_=kt_sb[0][:])
                    nc.sync.dma_start(out=d_qt[:], in_=qt_sb[0][:])

                # filler queue with prerequisites encoded by position:
                #   pair 0 steps: rest of K0, V quarter 0 (1/step), K1/Q1
                #   pair 1 steps: V quarter 1 (1/step), K2/Q2
                #   pairs 2-3:    V quarter 2 (1/2-step), K3/Q3, K4/Q4
                #   pairs 4-5:    V quarter 3 (1/2-step), K5-K7/Q5-Q7
                def kq_pair(p, prefetch=True):
                    out = (
                        [lambda p=p: prefetch_wk(p), lambda p=p: prefetch_wq(p)]
                        if prefetch
                        else []
                    )
                    for g in range(4):
                        out.append(lambda p=p, g=g: k_group(p, g))
                    for g in range(2):
                        out.append(lambda p=p, g=g: q_group(p, g))
                    return out

                def tp_half(p, half):
                    # PE re-transpose of normalized flipped heads (in the
                    # dead kt tile) into pair-stacked headsT in the qt tile
                    pjt = pjsum.tile([128, 512], F32, tag="pj", name="tp")
                    view = pjt[:].rearrange("p (a b) -> p a b", b=128)
                    for h in range(2):
                        for qi in range(4):
                            qc = half * 4 + qi
                            nc.tensor.matmul(
                                view[h * 64 : h * 64 + 64, qi, :],
                                o_norm[p][:, h * N_QC + qc, :],
                                ident[:],
                                start=True,
                                stop=True,
                                skip_group_check=True,
                            )
                    nc.vector.tensor_copy(
                        qt_sb[p][:, half * 512 : (half + 1) * 512], pjt[:]
                    )

                def tp_pair(p):
                    tp_half(p, 0)
                    tp_half(p, 1)

                # staging slots for phase-3 partials (f32 views of dead kt
                # space: upper halves of kt 0-6, lower half of kt 0)
                def stage_slot(qc):
                    if qc < 7:
                        return kt_sb[qc][:].bitcast(F32)[:, 512:1024]
                    return kt_sb[0][:].bitcast(F32)[:, 0:512]

                def ph3_partial(qc):
                    # sum over pairs 0-6 for (qc, dmc=0); pair 7 + the add
                    # complete it in the tail
                    pj = pjsum.tile([128, 512], F32, tag="pj", name="p3")
                    for p2 in range(NP - 1):
                        nc.tensor.matmul(
                            pj[:],
                            qt_sb[p2][:, qc * 128 : (qc + 1) * 128],
                            wo0_sb[:, p2, :],
                            start=(p2 == 0),
                            stop=(p2 == NP - 2),
                        )
                    nc.vector.tensor_copy(stage_slot(qc), pj[:])

                def v_quarter(q):
                    return [
                        lambda sc=sc, q=q: v_group(sc, q) for sc in range(N_SC)
                    ]

                # per-pair filler schedules: list of lists (one per step)
                def spread(items, nsteps):
                    # distribute items across nsteps as evenly as possible
                    outl = [[] for _ in range(nsteps)]
                    for i, it in enumerate(items):
                        outl[(i * nsteps) // len(items)].append(it)
                    return outl

                sched = {}
                vq = [v_quarter(q) for q in range(4)]
                # pair 0: v quarter 0 must run at 1/step (o of (pair0, sc)
                # needs it by step sc+1); remaining K0 columns by step
                # 4*g-1; K1/Q1 anywhere inside pair 0.
                sched[0] = [[vq[0][0], vq[0][1], lambda: k_group(0, 1)]] + [
                    [vq[0][sc + 1]] for sc in range(1, N_SC - 1)
                ] + [[]]
                extras0 = [
                    lambda: k_group(0, 2),
                    lambda: k_group(0, 3),
                ] + kq_pair(1, prefetch=False)
                for i, f in enumerate(extras0):
                    sched[0][3 + i].append(f)
                # v quarter q's chunk sc is first consumed by pair 2q at
                # its step sc+1, so each quarter's second half can lag into
                # the consuming pair itself; this spreads filler evenly and
                # leaves only the transposes for pair 7's steps.
                sched[1] = spread(vq[1][:8] + kq_pair(2), N_SC)
                sched[2] = spread(vq[1][8:] + kq_pair(3), N_SC)
                sched[3] = spread(vq[2][:8] + kq_pair(4), N_SC)
                sched[4] = spread(vq[2][8:] + kq_pair(5), N_SC)
                sched[5] = spread(vq[3][:8] + kq_pair(6), N_SC)
                sched[6] = spread(vq[3][8:] + kq_pair(7), N_SC)
                sched[7] = spread(
                    [lambda p=p: tp_half(p, 0) for p in range(7)]
                    + [lambda qc=qc: ph3_partial(qc) for qc in range(2)]
                    + [lambda p=p: tp_half(p, 1) for p in range(7)]
                    + [lambda qc=qc: ph3_partial(qc) for qc in range(2, 4)],
                    N_SC,
                )

                # o accumulator slot -> AP. 16 slots (h,qc) packed into PSUM
                # banks as 7+7+2 (a [128,8,65] f32 tile would straddle a 2KB
                # bank boundary).
                def o_slot(tiles, h, qc):
                    s = h * N_QC + qc
                    if s < 7:
                        return tiles[0][:, s, :]
                    if s < 14:
                        return tiles[1][:, s - 7, :]
                    return tiles[2][:, s - 14, :]

                def emit_o(p, sc, h, o_ps, e_tiles):
                    e_sb = e_tiles.pop((sc, h))
                    for qc in range(N_QC):
                        s = h * N_QC + qc
                        # start_tensor_calc marks the whole 2KB PSUM bank
                        # pending-zero, so only the FIRST slot of each bank
                        # may set it; the other slots' first write then
                        # lands on pending-zero bytes (= overwrite).
                        nc.tensor.matmul(
                            o_slot(o_ps, h, qc),
                            e_sb[:, qc * 128 : (qc + 1) * 128],
                            v_all[:, sc, 2 * p + h, :],
                            start=(sc == 0 and s in (0, 7, 14)),
                            stop=(sc == N_SC - 1),
                            skip_group_check=True,
                        )

                # ---------------- attention (phase 2) ----------------
                for p in range(NP if 2 in phases else 0):
                    if dbg and p == 1:
                        nc.sync.dma_start(out=d_onorm[:], in_=kt_sb[0][:, 0:1024])
                    if dbg and p == 6:
                        nc.sync.dma_start(
                            out=d_vall[:],
                            in_=v_all[:].rearrange("p a b c -> p (a b c)"),
                        )
                    if p == 7:
                        # V work is done; swap the vt/wv staging space for
                        # the first W_O half so phase-3 partial sums over
                        # pairs 0-6 can run as pair-7 filler
                        ves.close()
                        wo0 = les.enter_context(
                            tc.tile_pool(name="wo0", bufs=1)
                        )
                        wo0_sb = wo0.tile([128, NP, 512], BF16, tag="wo0")
                        nc.sync.dma_start(
                            out=wo0_sb[:], in_=w_o[:, :, 0:512]
                        )
                    o_ps = [
                        opsum.tile([128, 7, 65], F32, tag="oA", name="oA"),
                        opsum.tile([128, 7, 65], F32, tag="oB", name="oB"),
                        opsum.tile([128, 2, 65], F32, tag="oC", name="oC"),
                    ]
                    e_tiles = {}
                    for sc in range(N_SC):
                        for h in range(2):
                            lo, hi = h * 64, h * 64 + 64
                            sp = spsum.tile([128, QH], F32, tag=f"sp{h}")
                            for qc2 in range(QH // 512):
                                nc.tensor.matmul(
                                    sp[:, qc2 * 512 : (qc2 + 1) * 512],
                                    kt_sb[p][lo:hi, sc * 128 : (sc + 1) * 128],
                                    qt_sb[p][lo:hi, qc2 * 512 : (qc2 + 1) * 512],
                                    start=True,
                                    stop=True,
                                    skip_group_check=True,
                                )
                            e_sb = epool.tile([128, QH], BF16, tag=f"e{h}")
                            nc.scalar.activation(
                                e_sb[:],
                                sp[:],
                                mybir.ActivationFunctionType.Exp,
                                scale=0.125,
                            )
                            e_tiles[(sc, h)] = e_sb
                        # value matmuls lag two s-chunks behind the scores so
                        # ACT has time to produce e without stalling PE
                        if sc > 1:
                            for h in range(2):
                                emit_o(p, sc - 2, h, o_ps, e_tiles)
                        for fill in sched[p][sc]:
                            fill()
                    for h in range(2):
                        emit_o(p, N_SC - 2, h, o_ps, e_tiles)
                    emit_o(p, N_SC - 1, 0, o_ps, e_tiles)
                    # copy PSUM -> SBUF (bf16) + f32 denominators; the oA
                    # bank only holds h=0 slots, so it can drain while the
                    # h=1 value matmuls still run
                    o_sb = npool.tile([128, 16, 65], BF16, tag="osb")
                    den = npool.tile([128, 16], F32, tag="den")
                    nc.vector.tensor_copy(o_sb[:, 0:7, :], o_ps[0][:])
                    nc.vector.tensor_copy(den[:, 0:7, None], o_ps[0][:, :, 0:1])
                    emit_o(p, N_SC - 1, 1, o_ps, e_tiles)
                    nc.vector.tensor_copy(o_sb[:, 7:14, :], o_ps[1][:])
                    nc.vector.tensor_copy(o_sb[:, 14:16, :], o_ps[2][:])
                    nc.vector.tensor_copy(den[:, 7:14, None], o_ps[1][:, :, 0:1])
                    nc.vector.tensor_copy(den[:, 14:16, None], o_ps[2][:, :, 0:1])
                    rec = npool.tile([128, 16], F32, tag="rec")
                    nc.vector.reciprocal_approx_fast(rec[:], den[:])
                    nc.vector.tensor_mul(
                        o_norm[p][:],
                        o_sb[:, :, 1:65],
                        rec[:, :, None].broadcast_to([128, 16, 64]),
                    )

                if 2 in phases:
                    tp_pair(7)
                les.close()
                if dbg:
                    nc.sync.dma_start(out=d_heads[:], in_=qt_sb[0][:])

                # drain unused fillers (for phases subsets)
                if 2 not in phases:
                    for p in range(NP):
                        for step in sched.get(p, []):
                            for fill in step:
                                fill()

            # ------------- head re-transpose + output proj -------------
            with (
                tc.tile_pool(name="psum_f", bufs=4, space="PSUM") as fpsum,
                tc.tile_pool(name="wo", bufs=1) as wop,
                tc.tile_pool(name="fout", bufs=4) as fout,
            ):
                wo_sb = wop.tile([128, NP, DM], BF16, tag="wo")
                nc.sync.dma_start(out=wo_sb[:, :, 0:512], in_=w_o[:, :, 0:512])
                nc.sync.dma_start(
                    out=wo_sb[:, :, 512:1024], in_=w_o[:, :, 512:1024]
                )

                for qc in range(N_QC if 3 in phases else 0):
                    for dmc in range(DM // 512):
                        fp = fpsum.tile([128, 512], F32, tag="fp")
                        partial = dmc == 0 and qc < 4 and 2 in phases
                        for p in ([7] if partial else range(NP)):
                            nc.tensor.matmul(
                                fp[:],
                                qt_sb[p][:, qc * 128 : (qc + 1) * 128],
                                wo_sb[:, p, dmc * 512 : (dmc + 1) * 512],
                                start=(p == 7 if partial else p == 0),
                                stop=(p == NP - 1),
                            )
                        fo = fout.tile([128, 512], F32, tag="fo")
                        if partial:
                            nc.vector.tensor_add(fo[:], fp[:], stage_slot(qc))
                        else:
                            nc.scalar.copy(fo[:], fp[:])
                        nc.sync.dma_start(
                            out=out[
                                qc * 128 : (qc + 1) * 128,
                                dmc * 512 : (dmc + 1) * 512,
                            ],
                            in_=fo[:],
                        )
    nc.compile()
    return nc


_NC_CACHE = {}


def _get_nc():
    if "nc" not in _NC_CACHE:
        _NC_CACHE["nc"] = build()
    return _NC_CACHE["nc"]


def _prep_w3(w):
    # [H, DM, D] -> [mi=128, mo=8, (h d)=1024], bf16
    return np.ascontiguousarray(
        w.transpose(1, 0, 2).reshape(N_MO, 128, H * D).transpose(1, 0, 2)
    ).astype(ml_dtypes.bfloat16)


def _prep_w3p(w):
    # pair-major: [pair, mi=128, mo=8, 128]
    w3 = w.transpose(1, 0, 2).reshape(N_MO, 128, H * D).transpose(1, 0, 2)
    return np.ascontiguousarray(
        w3.reshape(128, N_MO, NP, 128).transpose(2, 0, 1, 3)
    ).astype(ml_dtypes.bfloat16)


def _prep_wo(w):
    # [H*D=1024, DM] -> [mi=128, chunk=8, DM], bf16
    return np.ascontiguousarray(
        w.reshape(NP, 128, DM).transpose(1, 0, 2)
    ).astype(ml_dtypes.bfloat16)


def _prep_xt(x):
    # [rows, DM] -> transposed [mo=8, 128, rows], bf16
    return np.ascontiguousarray(x.T.reshape(N_MO, 128, x.shape[0])).astype(
        ml_dtypes.bfloat16
    )


def _prep_vt(x):
    # [S, DM] -> [sc=16, 128(dm within mo), mo=8, 128(s within chunk)]
    # vt[sc, p, mo, c] = x[sc*128+c, mo*128+p]
    return np.ascontiguousarray(
        x.reshape(N_SC, 128, N_MO, 128).transpose(0, 3, 2, 1)
    ).astype(ml_dtypes.bfloat16)


def kernel(Q, K, V, W_Q, W_K, W_V, W_O, _trace=False):
    Q = np.asarray(Q, dtype=np.float32)
    K = np.asarray(K, dtype=np.float32)
    V = np.asarray(V, dtype=np.float32)
    wq = _prep_w3p(np.asarray(W_Q, dtype=np.float32))
    wk = _prep_w3p(np.asarray(W_K, dtype=np.float32))
    wv = _prep_w3(np.asarray(W_V, dtype=np.float32))
    wo = _prep_wo(np.asarray(W_O, dtype=np.float32))

    kt_b = [_prep_xt(K[b]) for b in range(B)]
    vt_b = [_prep_vt(V[b]) for b in range(B)]

    in_maps = []
    for c in range(N_CORES):
        b, half = c // 2, c % 2
        in_maps.append(
            {
                "QT": _prep_xt(Q[b, half * QH : (half + 1) * QH]),
                "KT": kt_b[b],
                "VTs": vt_b[b],
                "WQP": wq,
                "WKP": wk,
                "WV3": wv,
                "WO3": wo,
            }
        )

    nc = _get_nc()
    res = run_bass_kernel_spmd(nc, in_maps, list(range(N_CORES)), trace=_trace)
    out = np.empty((B, S, DM), dtype=np.float32)
    for c in range(N_CORES):
        b, half = c // 2, c % 2
        out[b, half * QH : (half + 1) * QH] = res.results[c]["out"]
    if _trace:
        kernel._last_results = res
    return out



# revision 7
# speedup vs baseline: 1.0737x; 1.0737x over previous
"""Multi-head attention (B=4, S=2048, H=16, d_model=1024, d_k=d_v=64) on 8
Trainium2 NeuronCores.

Sharding (v2): 8 cores = 4 batches x 2 head-halves (tensor-parallel over
heads, per the W_Q/W_K/W_V head-split + W_O row-split scheme). Each core
computes 8 heads (4 pairs) over the FULL query range S=2048 for its batch,
projects K/V only for its own heads (no duplicated projection work), runs
its partial output projection against its W_O row block, and the host sums
the two partial outputs per batch (the all-reduce).

Host prep: Q/K/V transposed to [d_model, seq] (V blocked by s-chunk) and
cast to bf16, as are all weights; W_Q/W_K pair-major; W_V/W_O sliced per
head-half.

Per-core pipeline: 8 attention units (pair p, query-half g), pair-major
order. Per unit, the baseline-style software pipeline: scoresT chunk =
kt.T @ qt into PSUM, e = exp(s/8) on ACT (the pacing engine), flipped
value matmul o[q, 65] = e-chunk.T @ [1|v] accumulated over s-chunks with
column 0 the softmax denominator. K/Q/V projection groups and the
output-projection partials run as PE filler inside the ACT-paced loop.

Normalized heads are written qc-major into the unit's dead qt half, then a
single DMA xbar transpose per unit produces pair-stacked headsT in the dead
ktx staging area (no PE transposes). Output projection: partial chains over
pairs 0-2 are staged to SBUF f32 (dead qtx space) as filler; pair-3 tail
matmul + DVE/GPSIMD add completes each chunk (g0 chunks inside unit 7,
g1 chunks in the drain tail).

PSUM note: start_tensor_calc marks the whole 2KB bank pending-zero, so
interleaved per-slot accumulation groups sharing a bank must issue exactly
one start (first slot); the other slots' first writes land on pending-zero
bytes, which the hardware treats as overwrite.
"""

import contextlib
import os
import sys

for _p in ("/opt/trn_rl_repo", "/root/.axon_site/_ro/trn_rl_repo"):
    if os.path.isdir(_p) and _p not in sys.path:
        sys.path.insert(0, _p)

import numpy as np
import ml_dtypes

import concourse.bass as bass  # noqa: F401
import concourse.tile as tile
from concourse import bacc, mybir
from concourse.bass_utils import run_bass_kernel_spmd

F32 = mybir.dt.float32
BF16 = mybir.dt.bfloat16

B, S, DM = 4, 2048, 1024
H, D = 16, 64
N_CORES = 8
NPC = 4  # head pairs per core (8 heads)
N_SC = S // 128  # kv 128-chunks
N_MO = DM // 128  # model-dim 128-chunks
N_G = 2  # query halves per core
QW = 1024  # query width per attention unit


def build(n_cores=N_CORES, phases=(1, 2, 3), dbg=False):
    nc = bacc.Bacc("TRN2", target_bir_lowering=False, debug=False, num_devices=n_cores)

    # host-transposed activations, bf16 (full batch; core's own head slice
    # of the weights)
    qt_d = nc.dram_tensor("QT", [N_MO, 128, S], BF16, kind="ExternalInput").ap()
    kt_d = nc.dram_tensor("KT", [N_MO, 128, S], BF16, kind="ExternalInput").ap()
    vt_d = nc.dram_tensor(
        "VTs", [N_SC, 128, N_MO, 128], BF16, kind="ExternalInput"
    ).ap()
    # pair-major: [pair, mi=128, mo=8, 128]
    w_q = nc.dram_tensor("WQP", [NPC, 128, N_MO, 128], BF16, kind="ExternalInput").ap()
    w_k = nc.dram_tensor("WKP", [NPC, 128, N_MO, 128], BF16, kind="ExternalInput").ap()
    w_v = nc.dram_tensor("WV3", [128, N_MO, 8 * D], BF16, kind="ExternalInput").ap()
    # [mi=128, pair=4, dm=1024]
    w_o = nc.dram_tensor("WO3", [128, NPC, DM], BF16, kind="ExternalInput").ap()
    out = nc.dram_tensor("out", [S, DM], F32, kind="ExternalOutput").ap()
    if dbg:
        d_kt = nc.dram_tensor("d_kt", [128, S], BF16, kind="ExternalOutput").ap()
        d_qt = nc.dram_tensor("d_qt", [128, S], BF16, kind="ExternalOutput").ap()
        d_vall = nc.dram_tensor(
            "d_vall", [128, N_SC * 8 * 65], BF16, kind="ExternalOutput"
        ).ap()
        d_onorm = nc.dram_tensor("d_onorm", [128, QW], BF16, kind="ExternalOutput").ap()
        d_heads = nc.dram_tensor("d_heads", [128, S], BF16, kind="ExternalOutput").ap()

    with tile.TileContext(nc) as tc:
        with (
            tc.tile_pool(name="pers", bufs=1) as pers,
            tc.tile_pool(name="wkq", bufs=2) as wkq,
            tc.tile_pool(name="wop", bufs=1) as wop,
        ):
            wo_sb = wop.tile([128, NPC, DM], BF16, tag="wo", name="wo")
            # v resident: per s-chunk block of 8 head-slots [1|v] (65 wide)
            v_all = pers.tile([128, N_SC, 8, 65], BF16, tag="v_all")
            nc.vector.memset(v_all[:, :, :, 0:1], 1.0)
            # kt[p]: pair-stacked [2*64, S]; qt[p]: [128, S], whose g-halves
            # are later reused for normalized flipped heads (qc-major)
            kt_sb = [
                pers.tile([128, S], BF16, tag=f"kt{p}", name=f"kt{p}")
                for p in range(NPC)
            ]
            qt_sb = [
                pers.tile([128, S], BF16, tag=f"qt{p}", name=f"qt{p}")
                for p in range(NPC)
            ]
            # resident staged transposed inputs for projections; ktx rows
            # double as headsT homes (mo-slot p holds pair p's headsT) once
            # the K projections have consumed them
            ktx = pers.tile([128, N_MO, S], BF16, tag="ktx")
            qtx = pers.tile([128, N_MO * S], BF16, tag="qtx")

            def heads_home(p):
                # pairs 0-2: own kt tile (dead after unit 2p+1's scores);
                # pair 3: ktx slot 3 (dead once the last K projection group
                # has run, i.e. from unit 6 on)
                return kt_sb[p] if p < NPC - 1 else ktx[:, NPC - 1, :]

            def heads_dst(p, g):
                # [128, 8, 128] chunked-transpose target
                return heads_home(p)[:, g * QW : (g + 1) * QW].rearrange(
                    "p (c q) -> p c q", q=128
                )

            def heads_chunk(p, qc):
                # out-projection stationary: [128 hv, 128 q] for global qc
                return heads_home(p)[:, qc * 128 : (qc + 1) * 128]

            def stage_slot(i):
                # 16 f32 staging slots in dead qtx space
                return qtx[:].bitcast(F32)[:, i * 512 : (i + 1) * 512]

            with (
                tc.tile_pool(name="psum_sp", bufs=1, space="PSUM") as spsum,
                tc.tile_pool(name="psum_o", bufs=1, space="PSUM") as opsum,
                tc.tile_pool(name="psum_pj", bufs=1, space="PSUM") as pjsum,
                tc.tile_pool(name="epool", bufs=3) as epool,
                tc.tile_pool(name="npool", bufs=2) as npool,
                tc.tile_pool(name="fout", bufs=3) as fout,
            ):
                ves = contextlib.ExitStack()
                vpool = ves.enter_context(tc.tile_pool(name="vpool", bufs=1))
                vtx = [
                    vpool.tile([128, N_MO, 128], BF16, tag=f"vt{sc}", name=f"vt{sc}")
                    for sc in range(N_SC)
                ]
                wv_sb = vpool.tile([128, N_MO, 8 * D], BF16, tag="wv")

                cur_wk, cur_wq = {}, {}

                def prefetch_wk(p):
                    wkt = wkq.tile([128, N_MO, 128], BF16, tag="wk", name="wk")
                    cur_wk[p] = wkt
                    nc.sync.dma_start(out=wkt[:], in_=w_k[p])

                def prefetch_wq(p):
                    wqt = wkq.tile([128, N_MO, 128], BF16, tag="wq", name="wq")
                    cur_wq[p] = wqt
                    nc.sync.dma_start(out=wqt[:], in_=w_q[p])

                def dma_ktx(g):
                    for mo in range(N_MO):
                        nc.sync.dma_start(
                            out=ktx[:, mo, g * 512 : (g + 1) * 512],
                            in_=kt_d[mo, :, g * 512 : (g + 1) * 512],
                        )

                def dma_qtx(g):
                    for mo in range(N_MO):
                        nc.sync.dma_start(
                            out=qtx[:, mo * S + g * 512 : mo * S + (g + 1) * 512],
                            in_=qt_d[mo, :, g * 512 : (g + 1) * 512],
                        )

                def dma_wv(hp):
                    nc.sync.dma_start(
                        out=wv_sb[:, :, hp * 128 : (hp + 1) * 128],
                        in_=w_v[:, :, hp * 128 : (hp + 1) * 128],
                    )

                def dma_vt(sc):
                    nc.sync.dma_start(out=vtx[sc][:], in_=vt_d[sc])

                # startup-critical DMA order: pair-0 K/Q weights + first K/Q
                # columns, then V / remaining K/Q interleaved
                prefetch_wk(0)
                prefetch_wq(0)
                dma_ktx(0)
                dma_qtx(0)
                dma_qtx(1)
                dma_wv(0)
                dma_wv(1)
                dma_vt(0)
                dma_vt(1)
                dma_vt(2)
                dma_ktx(1)
                dma_vt(3)
                dma_vt(4)
                dma_ktx(2)
                dma_vt(5)
                dma_vt(6)
                dma_ktx(3)
                prefetch_wk(1)
                prefetch_wq(1)
                dma_vt(7)
                dma_wv(2)
                for sc in range(8, N_SC):
                    dma_vt(sc)
                dma_wv(3)
                dma_qtx(2)
                dma_qtx(3)

                # ---------- projection "filler" groups ----------
                def k_group(p, g):
                    pj = pjsum.tile([128, 512], F32, tag="pj")
                    for mo in range(N_MO):
                        nc.tensor.matmul(
                            pj[:],
                            cur_wk[p][:, mo, :],
                            ktx[:, mo, g * 512 : (g + 1) * 512],
                            start=(mo == 0),
                            stop=(mo == N_MO - 1),
                        )
                    nc.vector.tensor_copy(kt_sb[p][:, g * 512 : (g + 1) * 512], pj[:])

                def q_group(p, g):
                    pj = pjsum.tile([128, 512], F32, tag="pj")
                    for mo in range(N_MO):
                        nc.tensor.matmul(
                            pj[:],
                            cur_wq[p][:, mo, :],
                            qtx[:, mo * S + g * 512 : mo * S + (g + 1) * 512],
                            start=(mo == 0),
                            stop=(mo == N_MO - 1),
                        )
                    nc.vector.tensor_copy(qt_sb[p][:, g * 512 : (g + 1) * 512], pj[:])

                def v_group(sc, hp):
                    # one pair's two head-slots (128 wv cols), chunk sc
                    pj = pjsum.tile([128, 512], F32, tag="pj")
                    for mo in range(N_MO):
                        nc.tensor.matmul(
                            pj[:, 0:128],
                            vtx[sc][:, mo, :],
                            wv_sb[:, mo, hp * 128 : (hp + 1) * 128],
                            start=(mo == 0),
                            stop=(mo == N_MO - 1),
                        )
                    nc.vector.tensor_copy(
                        v_all[:, sc, 2 * hp : 2 * hp + 2, 1:65],
                        pj[:, 0:128].rearrange("p (h w) -> p h w", h=2),
                    )

                # out-projection partial chains over pairs 0-2, staged to
                # SBUF f32; pair-3 tail matmul + add completes a chunk.
                # slot index i in [0,16): g0 uses (qc*2+dmc), g1 reuses the
                # same slots after the g0 tails have consumed them.
                def stage(qc, dmc):
                    pj = pjsum.tile([128, 512], F32, tag="pj", name="st")
                    for p2 in range(NPC - 1):
                        nc.tensor.matmul(
                            pj[:],
                            heads_chunk(p2, qc),
                            wo_sb[:, p2, dmc * 512 : (dmc + 1) * 512],
                            start=(p2 == 0),
                            stop=(p2 == NPC - 2),
                        )
                    nc.vector.tensor_copy(stage_slot((qc % 8) * 2 + dmc), pj[:])

                def tail(qc, dmc, pool, even):
                    tl = pool.tile([128, 512], F32, tag="pj", name="tl")
                    nc.tensor.matmul(
                        tl[:],
                        heads_chunk(NPC - 1, qc),
                        wo_sb[:, NPC - 1, dmc * 512 : (dmc + 1) * 512],
                        start=True,
                        stop=True,
                    )
                    fo = fout.tile([128, 512], F32, tag="fo")
                    nc.vector.tensor_add(fo[:], tl[:], stage_slot((qc % 8) * 2 + dmc))
                    nc.sync.dma_start(
                        out=out[qc * 128 : (qc + 1) * 128, dmc * 512 : (dmc + 1) * 512],
                        in_=fo[:],
                    )

                # K/Q for unit (0,0) before attention starts
                k_group(0, 0)
                q_group(0, 0)
                q_group(0, 1)

                if dbg:
                    nc.sync.dma_start(out=d_kt[:], in_=kt_sb[0][:])
                    nc.sync.dma_start(out=d_qt[:], in_=qt_sb[0][:])

                def spread(items, nsteps=N_SC):
                    outl = [[] for _ in range(nsteps)]
                    for i, it in enumerate(items):
                        outl[(i * nsteps) // len(items)].append(it)
                    return outl

                def K(p, g):
                    return lambda: k_group(p, g)

                def Q(p, g):
                    return lambda: q_group(p, g)

                def V(sc, hp):
                    return lambda: v_group(sc, hp)

                def ST(qc, dmc):
                    return lambda: stage(qc, dmc)

                def TL(qc, dmc, even):
                    return lambda: tail(qc, dmc, pjsum, even)

                def PF(p):
                    return [lambda p=p: prefetch_wk(p), lambda p=p: prefetch_wq(p)]

                # per-unit filler schedules (unit = 2*p + g, pair-major)
                sched = {}
                # u0: v pair-0 slots (1/step, just in time), rest of K0, K1
                sched[0] = [[V(0, 0), V(1, 0), K(0, 1)]] + [
                    [V(sc + 1, 0)] for sc in range(1, N_SC - 1)
                ] + [[]]
                extras0 = [K(0, 2), K(0, 3), Q(0, 2), Q(0, 3)] + PF(1)
                for i, f in enumerate(extras0):
                    sched[0][3 + i].append(f)
                # u1: v pair-1 slots, K1 groups, Q1 g0/g1
                sched[1] = spread(
                    [V(sc, 1) for sc in range(N_SC)]
                    + [K(1, g) for g in range(4)]
                    + [Q(1, 0), Q(1, 1)]
                    + PF(2)
                )
                # u2: K2, Q1 g2/g3, start v pair-2
                sched[2] = spread(
                    [V(sc, 2) for sc in range(8)]
                    + [K(2, g) for g in range(4)]
                    + [Q(1, 2), Q(1, 3)]
                )
                # u3: rest v pair-2, Q2 g0/g1, start K3
                sched[3] = spread(
                    PF(3)
                    + [V(sc, 2) for sc in range(8, N_SC)]
                    + [Q(2, 0), Q(2, 1)]
                    + [K(3, 0)]
                )
                # u4: v pair-3, Q2 g2/g3, K3
                sched[4] = spread(
                    [V(sc, 3) for sc in range(N_SC)] + [Q(2, 2), Q(2, 3)] + [K(3, 1)]
                )
                # u5: K3 rest, Q3 all
                sched[5] = spread(
                    [K(3, 2), K(3, 3)] + [Q(3, g) for g in range(4)]
                )
                # u6: g0 out-proj partial chains (pairs 0-2)
                sched[6] = spread(
                    [ST(qc, dmc) for qc in range(8) for dmc in range(2)]
                )
                # u7: g0 tails + g1 partial chains, interleaved so each g1
                # stage reuses the slot its paired g0 tail just freed
                u7 = [[] for _ in range(N_SC)]
                for s in range(N_SC):
                    qc, dmc = s // 2, s % 2
                    u7[s].append(TL(qc, dmc, s % 2 == 0))
                    u7[s].append(ST(qc + 8, dmc))
                sched[7] = u7

                # o accumulator slot -> AP. 16 slots (h,qc) packed into PSUM
                # banks as 7+7+2 (bank-straddle constraint).
                def o_slot(tiles, h, qc):
                    s = h * 8 + qc
                    if s < 7:
                        return tiles[0][:, s, :]
                    if s < 14:
                        return tiles[1][:, s - 7, :]
                    return tiles[2][:, s - 14, :]

                def emit_o(p, sc, h, o_ps, e_tiles):
                    e_sb = e_tiles.pop((sc, h))
                    for qc in range(8):
                        s = h * 8 + qc
                        nc.tensor.matmul(
                            o_slot(o_ps, h, qc),
                            e_sb[:, qc * 128 : (qc + 1) * 128],
                            v_all[:, sc, 2 * p + h, :],
                            start=(sc == 0 and s in (0, 7, 14)),
                            stop=(sc == N_SC - 1),
                            skip_group_check=True,
                        )

                # ---------------- attention (phase 2) ----------------
                for u in range(2 * NPC if 2 in phases else 0):
                    p, g = u // 2, u % 2
                    if dbg and u == 2:
                        nc.sync.dma_start(out=d_onorm[:], in_=qt_sb[0][:, 0:QW])
                    if dbg and u == 6:
                        nc.sync.dma_start(
                            out=d_vall[:],
                            in_=v_all[:].rearrange("p a b c -> p (a b c)"),
                        )
                    if u == 6:
                        # V work done; release its staging space and load W_O
                        ves.close()
                        nc.sync.dma_start(out=wo_sb[:], in_=w_o[:])
                    o_ps = [
                        opsum.tile([128, 7, 65], F32, tag="oA", name="oA"),
                        opsum.tile([128, 7, 65], F32, tag="oB", name="oB"),
                        opsum.tile([128, 2, 65], F32, tag="oC", name="oC"),
                    ]
                    e_tiles = {}
                    for sc in range(N_SC):
                        for h in range(2):
                            lo, hi = h * 64, h * 64 + 64
                            sp = spsum.tile([128, QW], F32, tag=f"sp{h}")
                            for qc2 in range(QW // 512):
                                nc.tensor.matmul(
                                    sp[:, qc2 * 512 : (qc2 + 1) * 512],
                                    kt_sb[p][lo:hi, sc * 128 : (sc + 1) * 128],
                                    qt_sb[p][
                                        lo:hi,
                                        g * QW + qc2 * 512 : g * QW + (qc2 + 1) * 512,
                                    ],
                                    start=True,
                                    stop=True,
                                    skip_group_check=True,
                                )
                            e_sb = epool.tile([128, QW], BF16, tag=f"e{h}")
                            nc.scalar.activation(
                                e_sb[:],
                                sp[:],
                                mybir.ActivationFunctionType.Exp,
                                scale=0.125,
                            )
                            e_tiles[(sc, h)] = e_sb
                        # value matmuls lag two s-chunks behind the scores
                        if sc > 1:
                            for h in range(2):
                                emit_o(p, sc - 2, h, o_ps, e_tiles)
                        for fill in sched[u][sc]:
                            fill()
                    for h in range(2):
                        emit_o(p, N_SC - 2, h, o_ps, e_tiles)
                    emit_o(p, N_SC - 1, 0, o_ps, e_tiles)
                    # drain PSUM -> SBUF (bf16) + f32 denominators; oA only
                    # holds h=0 slots so it can drain while h=1 value
                    # matmuls still run
                    o_sb = npool.tile([128, 16, 65], BF16, tag="osb")
                    den = npool.tile([128, 16], F32, tag="den")
                    nc.vector.tensor_copy(o_sb[:, 0:7, :], o_ps[0][:])
                    nc.vector.tensor_copy(den[:, 0:7, None], o_ps[0][:, :, 0:1])
                    emit_o(p, N_SC - 1, 1, o_ps, e_tiles)
                    nc.vector.tensor_copy(o_sb[:, 7:14, :], o_ps[1][:])
                    nc.vector.tensor_copy(o_sb[:, 14:16, :], o_ps[2][:])
                    nc.vector.tensor_copy(den[:, 7:14, None], o_ps[1][:, :, 0:1])
                    nc.vector.tensor_copy(den[:, 14:16, None], o_ps[2][:, :, 0:1])
                    rec = npool.tile([128, 16], F32, tag="rec")
                    nc.vector.reciprocal_approx_fast(rec[:], den[:])
                    # normalized flipped heads, qc-major, into the dead qt
                    # half: col = qc*128 + h*64 + v
                    o_norm = qt_sb[p][:, g * QW : (g + 1) * QW].rearrange(
                        "p (a b c) -> p b a c", a=8, b=2, c=64
                    )
                    nc.vector.tensor_mul(
                        o_norm,
                        o_sb[:, :, 1:65].rearrange("p (h q) c -> p h q c", h=2),
                        rec[:].rearrange("p (h q) -> p h q", h=2)[
                            :, :, :, None
                        ].broadcast_to([128, 2, 8, 64]),
                    )
                    # xbar transposes -> pair-stacked headsT, emitted only
                    # once the destination is dead: pairs 0-2 overwrite their
                    # kt tile after unit 2p+1's last scores; pair 3 goes to
                    # ktx (g0 after unit 6, g1 after unit 7)
                    if p < NPC - 1 and g == 1:
                        for g2 in range(2):
                            nc.sync.dma_start_transpose(
                                out=heads_dst(p, g2),
                                in_=qt_sb[p][:, g2 * QW : (g2 + 1) * QW],
                            )
                    elif p == NPC - 1:
                        nc.sync.dma_start_transpose(
                            out=heads_dst(p, g),
                            in_=qt_sb[p][:, g * QW : (g + 1) * QW],
                        )

                if dbg:
                    nc.sync.dma_start(out=d_heads[:], in_=kt_sb[0][:])

                # drain unused fillers (for phases subsets)
                if 2 not in phases:
                    ves.close()
                    nc.sync.dma_start(out=wo_sb[:], in_=w_o[:])
                    for u in range(2 * NPC):
                        for step in sched.get(u, []):
                            for fill in step:
                                fill()

            # ---------------- g1 out-projection tails ----------------
            with tc.tile_pool(name="psum_t", bufs=4, space="PSUM") as tpsum:
                if 3 in phases:
                    with tc.tile_pool(name="fout2", bufs=4) as fout:
                        def tail2(qc, dmc, even):
                            tl = tpsum.tile([128, 512], F32, tag="tl")
                            nc.tensor.matmul(
                                tl[:],
                                heads_chunk(NPC - 1, qc),
                                wo_sb[:, NPC - 1, dmc * 512 : (dmc + 1) * 512],
                                start=True,
                                stop=True,
                            )
                            fo = fout.tile([128, 512], F32, tag="fo")
                            nc.vector.tensor_add(
                                fo[:], tl[:], stage_slot((qc % 8) * 2 + dmc)
                            )
                            nc.sync.dma_start(
                                out=out[
                                    qc * 128 : (qc + 1) * 128,
                                    dmc * 512 : (dmc + 1) * 512,
                                ],
                                in_=fo[:],
                            )

                        for s in range(N_SC):
                            tail2(8 + s // 2, s % 2, s % 2 == 0)
    nc.compile()
    return nc


_NC_CACHE = {}


def _get_nc():
    if "nc" not in _NC_CACHE:
        _NC_CACHE["nc"] = build()
    return _NC_CACHE["nc"]


def _prep_w3p(w):
    # [H, DM, D] -> pair-major [8 pairs, mi=128, mo=8, 128], bf16
    w3 = w.transpose(1, 0, 2).reshape(N_MO, 128, H * D).transpose(1, 0, 2)
    return np.ascontiguousarray(
        w3.reshape(128, N_MO, H // 2, 128).transpose(2, 0, 1, 3)
    ).astype(ml_dtypes.bfloat16)


def _prep_wv(w):
    # [H, DM, D] -> [mi=128, mo=8, (h d)=1024], bf16
    return np.ascontiguousarray(
        w.transpose(1, 0, 2).reshape(N_MO, 128, H * D).transpose(1, 0, 2)
    ).astype(ml_dtypes.bfloat16)


def _prep_wo(w):
    # [H*D=1024, DM] -> [mi=128, chunk=8, DM], bf16
    return np.ascontiguousarray(w.reshape(8, 128, DM).transpose(1, 0, 2)).astype(
        ml_dtypes.bfloat16
    )


def _prep_xt(x):
    # [S, DM] -> transposed [mo=8, 128, S], bf16
    return np.ascontiguousarray(x.T.reshape(N_MO, 128, x.shape[0])).astype(
        ml_dtypes.bfloat16
    )


def _prep_vt(x):
    # [S, DM] -> [sc=16, 128(dm within mo), mo=8, 128(s within chunk)]
    return np.ascontiguousarray(
        x.reshape(N_SC, 128, N_MO, 128).transpose(0, 3, 2, 1)
    ).astype(ml_dtypes.bfloat16)


def kernel(Q, K, V, W_Q, W_K, W_V, W_O, _trace=False):
    Q = np.asarray(Q, dtype=np.float32)
    K = np.asarray(K, dtype=np.float32)
    V = np.asarray(V, dtype=np.float32)
    wq8 = _prep_w3p(np.asarray(W_Q, dtype=np.float32))  # [8 pairs, ...]
    wk8 = _prep_w3p(np.asarray(W_K, dtype=np.float32))
    wv8 = _prep_wv(np.asarray(W_V, dtype=np.float32))  # [128, 8, 1024]
    wo8 = _prep_wo(np.asarray(W_O, dtype=np.float32))  # [128, 8, DM]

    qt_b = [_prep_xt(Q[b]) for b in range(B)]
    kt_b = [_prep_xt(K[b]) for b in range(B)]
    vt_b = [_prep_vt(V[b]) for b in range(B)]
    wq_h = [np.ascontiguousarray(wq8[hh * NPC : (hh + 1) * NPC]) for hh in range(2)]
    wk_h = [np.ascontiguousarray(wk8[hh * NPC : (hh + 1) * NPC]) for hh in range(2)]
    wv_h = [
        np.ascontiguousarray(wv8[:, :, hh * 512 : (hh + 1) * 512]) for hh in range(2)
    ]
    wo_h = [
        np.ascontiguousarray(wo8[:, hh * NPC : (hh + 1) * NPC, :]) for hh in range(2)
    ]

    in_maps = []
    for c in range(N_CORES):
        b, hh = c // 2, c % 2
        in_maps.append(
            {
                "QT": qt_b[b],
                "KT": kt_b[b],
                "VTs": vt_b[b],
                "WQP": wq_h[hh],
                "WKP": wk_h[hh],
                "WV3": wv_h[hh],
                "WO3": wo_h[hh],
            }
        )

    nc = _get_nc()
    res = run_bass_kernel_spmd(nc, in_maps, list(range(N_CORES)), trace=_trace)
    out = np.empty((B, S, DM), dtype=np.float32)
    for b in range(B):
        out[b] = res.results[2 * b]["out"] + res.results[2 * b + 1]["out"]
    if _trace:
        kernel._last_results = res
    return out


# revision 12
# speedup vs baseline: 1.1265x; 1.0491x over previous
"""Multi-head attention (B=4, S=2048, H=16, d_model=1024, d_k=d_v=64) on 8
Trainium2 NeuronCores.

Sharding (v2): 8 cores = 4 batches x 2 head-halves (tensor-parallel over
heads, per the W_Q/W_K/W_V head-split + W_O row-split scheme). Each core
computes 8 heads (4 pairs) over the FULL query range S=2048 for its batch,
projects K/V only for its own heads (no duplicated projection work), runs
its partial output projection against its W_O row block, and the host sums
the two partial outputs per batch (the all-reduce).

Host prep: Q/K/V transposed to [d_model, seq] (V blocked by s-chunk) and
cast to bf16, as are all weights; W_Q/W_K pair-major; W_V/W_O sliced per
head-half.

Per-core pipeline: 8 attention units (pair p, query-half g), pair-major
order. Per unit, the baseline-style software pipeline: scoresT chunk =
kt.T @ qt into PSUM, e = exp(s/8) on ACT (the pacing engine), flipped
value matmul o[q, 65] = e-chunk.T @ [1|v] accumulated over s-chunks with
column 0 the softmax denominator. K/Q/V projection groups and the
output-projection partials run as PE filler inside the ACT-paced loop.

Normalized heads are written qc-major into the unit's dead qt half, then a
single DMA xbar transpose per unit produces pair-stacked headsT in the dead
ktx staging area (no PE transposes). Output projection: partial chains over
pairs 0-2 are staged to SBUF f32 (dead qtx space) as filler; pair-3 tail
matmul + DVE/GPSIMD add completes each chunk (g0 chunks inside unit 7,
g1 chunks in the drain tail).

PSUM note: start_tensor_calc marks the whole 2KB bank pending-zero, so
interleaved per-slot accumulation groups sharing a bank must issue exactly
one start (first slot); the other slots' first writes land on pending-zero
bytes, which the hardware treats as overwrite.
"""

import contextlib
import os
import sys

for _p in ("/opt/trn_rl_repo", "/root/.axon_site/_ro/trn_rl_repo"):
    if os.path.isdir(_p) and _p not in sys.path:
        sys.path.insert(0, _p)

import numpy as np
import ml_dtypes

import concourse.bass as bass  # noqa: F401
import concourse.tile as tile
from concourse import bacc, mybir
from concourse.bass_utils import run_bass_kernel_spmd
from concourse.masks import make_identity

F32 = mybir.dt.float32
F32R = mybir.dt.float32r
BF16 = mybir.dt.bfloat16

B, S, DM = 4, 2048, 1024
H, D = 16, 64
N_CORES = 8
NPC = 4  # head pairs per core (8 heads)
N_SC = S // 128  # kv 128-chunks
N_MO = DM // 128  # model-dim 128-chunks
N_G = 2  # query halves per core
QW = 1024  # query width per attention unit


def build(n_cores=N_CORES, phases=(1, 2, 3), dbg=False):
    nc = bacc.Bacc("TRN2", target_bir_lowering=False, debug=False, num_devices=n_cores)

    # host-transposed activations, bf16 (full batch; core's own head slice
    # of the weights)
    qt_d = nc.dram_tensor("QT", [128, N_MO, S], BF16, kind="ExternalInput").ap()
    kt_d = nc.dram_tensor("KT", [128, N_MO, S], BF16, kind="ExternalInput").ap()
    vt_d = nc.dram_tensor(
        "VTs", [128, N_SC, N_MO, 128], BF16, kind="ExternalInput"
    ).ap()
    # pair-major: [pair, mi=128, mo=8, 128]
    w_q = nc.dram_tensor("WQP", [NPC, 128, N_MO, 128], BF16, kind="ExternalInput").ap()
    w_k = nc.dram_tensor("WKP", [NPC, 128, N_MO, 128], BF16, kind="ExternalInput").ap()
    w_v = nc.dram_tensor("WV3", [128, N_MO, 8 * D], BF16, kind="ExternalInput").ap()
    # [mi=128, pair=4, dm=1024]
    w_o = nc.dram_tensor("WO3", [128, NPC, DM], BF16, kind="ExternalInput").ap()
    out = nc.dram_tensor("out", [S, DM], F32, kind="ExternalOutput").ap()
    if dbg:
        d_kt = nc.dram_tensor("d_kt", [128, S], BF16, kind="ExternalOutput").ap()
        d_qt = nc.dram_tensor("d_qt", [128, S], BF16, kind="ExternalOutput").ap()
        d_vall = nc.dram_tensor(
            "d_vall", [128, N_SC * 8 * 65], BF16, kind="ExternalOutput"
        ).ap()
        d_onorm = nc.dram_tensor("d_onorm", [128, QW], BF16, kind="ExternalOutput").ap()
        d_heads = nc.dram_tensor("d_heads", [128, S], BF16, kind="ExternalOutput").ap()

    with tile.TileContext(nc) as tc:
        with (
            tc.tile_pool(name="pers", bufs=1) as pers,
            tc.tile_pool(name="wkq", bufs=2) as wkq,
            tc.tile_pool(name="wop", bufs=1) as wop,
        ):
            wo_sb = wop.tile([128, NPC, DM], BF16, tag="wo", name="wo")
            ident_f32 = wop.tile([128, 128], F32, tag="identf", name="ident_f32")
            make_identity(nc, ident_f32[:])
            ident_bf = wop.tile([128, 128], BF16, tag="ident", name="ident_bf")
            nc.vector.tensor_copy(ident_bf[:], ident_f32[:])
            # v resident: per s-chunk block of 8 head-slots [1|v] (65 wide)
            v_all = pers.tile([128, N_SC, 8, 65], BF16, tag="v_all")
            nc.vector.memset(v_all[:, :, :, 0:1], 1.0)
            # kt[p]: pair-stacked [2*64, S]; qt[p]: [128, S], whose g-halves
            # are later reused for normalized flipped heads (qc-major)
            kt_sb = [
                pers.tile([128, S], BF16, tag=f"kt{p}", name=f"kt{p}")
                for p in range(NPC)
            ]
            qt_sb = [
                pers.tile([128, S], BF16, tag=f"qt{p}", name=f"qt{p}")
                for p in range(NPC)
            ]
            # resident staged transposed inputs for projections; ktx rows
            # double as headsT homes (mo-slot p holds pair p's headsT) once
            # the K projections have consumed them
            ktx = pers.tile([128, N_MO, S], BF16, tag="ktx")
            qtx = pers.tile([128, N_MO * S], BF16, tag="qtx")

            def heads_home(p):
                # pairs 0-2: own kt tile (dead after unit 2p+1's scores);
                # pair 3: ktx slot 3 (dead once the last K projection group
                # has run, i.e. from unit 6 on)
                return kt_sb[p] if p < NPC - 1 else ktx[:, NPC - 1, :]

            def heads_dst(p, g):
                # [128, 8, 128] chunked-transpose target
                return heads_home(p)[:, g * QW : (g + 1) * QW].rearrange(
                    "p (c q) -> p c q", q=128
                )

            def heads_chunk(p, qc):
                # out-projection stationary: [128 hv, 128 q] for global qc
                return heads_home(p)[:, qc * 128 : (qc + 1) * 128]

            def stage_slot(i):
                # 16 bf16 staging slots in dead qtx space (bf16 so the tail
                # identity-matmul re-feed stays a plain bf16 matmul)
                return qtx[:, i * 512 : (i + 1) * 512]

            with (
                tc.tile_pool(name="psum_sp", bufs=1, space="PSUM") as spsum,
                tc.tile_pool(name="psum_o", bufs=1, space="PSUM") as opsum,
                tc.tile_pool(name="psum_pj", bufs=1, space="PSUM") as pjsum,
                tc.tile_pool(name="epool", bufs=3) as epool,
                tc.tile_pool(name="npool", bufs=2) as npool,
                tc.tile_pool(name="fout", bufs=3) as fout,
            ):
                ves = contextlib.ExitStack()
                vpool = ves.enter_context(tc.tile_pool(name="vpool", bufs=1))
                vtx = vpool.tile([128, N_SC, N_MO, 128], BF16, tag="vtx")
                wv_sb = vpool.tile([128, N_MO, 8 * D], BF16, tag="wv")

                cur_wk, cur_wq = {}, {}

                def prefetch_wk(p):
                    wkt = wkq.tile([128, N_MO, 128], BF16, tag="wk", name="wk")
                    cur_wk[p] = wkt
                    nc.sync.dma_start(out=wkt[:], in_=w_k[p])

                def prefetch_wq(p):
                    wqt = wkq.tile([128, N_MO, 128], BF16, tag="wq", name="wq")
                    cur_wq[p] = wqt
                    nc.sync.dma_start(out=wqt[:], in_=w_q[p])

                def dma_ktx(g):
                    nc.sync.dma_start(
                        out=ktx[:, :, g * 512 : (g + 1) * 512],
                        in_=kt_d[:, :, g * 512 : (g + 1) * 512],
                    )

                def dma_qtx(g):
                    nc.sync.dma_start(
                        out=qtx[:].rearrange("p (m s) -> p m s", s=S)[
                            :, :, g * 512 : (g + 1) * 512
                        ],
                        in_=qt_d[:, :, g * 512 : (g + 1) * 512],
                    )

                def dma_wv():
                    nc.sync.dma_start(out=wv_sb[:], in_=w_v[:])

                def dma_vt(blk):
                    nc.sync.dma_start(
                        out=vtx[:, 4 * blk : 4 * blk + 4, :, :],
                        in_=vt_d[:, 4 * blk : 4 * blk + 4, :, :],
                    )

                # startup-critical DMA order: pair-0 K/Q weights + first K/Q
                # columns, then V / remaining K/Q interleaved
                prefetch_wk(0)
                prefetch_wq(0)
                dma_ktx(0)
                dma_qtx(0)
                dma_wv()
                dma_qtx(1)
                dma_vt(0)
                dma_ktx(1)
                dma_vt(1)
                dma_ktx(2)
                dma_ktx(3)
                dma_vt(2)
                dma_vt(3)
                dma_qtx(2)
                dma_qtx(3)

                # ---------- projection "filler" groups ----------
                def k_group(p, g):
                    pj = pjsum.tile([128, 512], F32, tag="pj")
                    for mo in range(N_MO):
                        nc.tensor.matmul(
                            pj[:],
                            cur_wk[p][:, mo, :],
                            ktx[:, mo, g * 512 : (g + 1) * 512],
                            start=(mo == 0),
                            stop=(mo == N_MO - 1),
                        )
                    nc.vector.tensor_copy(kt_sb[p][:, g * 512 : (g + 1) * 512], pj[:])

                def q_group(p, g):
                    pj = pjsum.tile([128, 512], F32, tag="pj")
                    for mo in range(N_MO):
                        nc.tensor.matmul(
                            pj[:],
                            cur_wq[p][:, mo, :],
                            qtx[:, mo * S + g * 512 : mo * S + (g + 1) * 512],
                            start=(mo == 0),
                            stop=(mo == N_MO - 1),
                        )
                    nc.vector.tensor_copy(qt_sb[p][:, g * 512 : (g + 1) * 512], pj[:])

                def v_group(sc, hp):
                    # one pair's two head-slots (128 wv cols), chunk sc
                    pj = pjsum.tile([128, 512], F32, tag="pj")
                    for mo in range(N_MO):
                        nc.tensor.matmul(
                            pj[:, 0:128],
                            vtx[:, sc, mo, :],
                            wv_sb[:, mo, hp * 128 : (hp + 1) * 128],
                            start=(mo == 0),
                            stop=(mo == N_MO - 1),
                        )
                    nc.vector.tensor_copy(
                        v_all[:, sc, 2 * hp : 2 * hp + 2, 1:65],
                        pj[:, 0:128].rearrange("p (h w) -> p h w", h=2),
                    )

                # out-projection:
                #  - g1 chunks: partial chains over pairs 0-2 staged to SBUF
                #    f32 (dead qtx space) inside units 6-7; the drain tail
                #    re-feeds each staged partial into PSUM with an identity
                #    matmul (float32r moving operand: full rate, ~tf32
                #    rounding) on top of the pair-3 matmul, then DMAs the
                #    PSUM result out directly.
                #  - g0 chunks: full 4-pair chains inside unit 7.
                def stage(qc, dmc):
                    pj = pjsum.tile([128, 512], F32, tag="pj", name="st")
                    for p2 in range(NPC - 1):
                        nc.tensor.matmul(
                            pj[:],
                            heads_chunk(p2, qc),
                            wo_sb[:, p2, dmc * 512 : (dmc + 1) * 512],
                            start=(p2 == 0),
                            stop=(p2 == NPC - 2),
                        )
                    nc.vector.tensor_copy(stage_slot((qc - 8) * 2 + dmc), pj[:])

                def full_chain(qc, dmc):
                    pj = pjsum.tile([128, 512], F32, tag="pj", name="fc")
                    for p2 in range(NPC):
                        nc.tensor.matmul(
                            pj[:],
                            heads_chunk(p2, qc),
                            wo_sb[:, p2, dmc * 512 : (dmc + 1) * 512],
                            start=(p2 == 0),
                            stop=(p2 == NPC - 1),
                        )
                    fo = fout.tile([128, 512], F32, tag="fo")
                    nc.vector.tensor_copy(fo[:], pj[:])
                    nc.sync.dma_start(
                        out=out[qc * 128 : (qc + 1) * 128, dmc * 512 : (dmc + 1) * 512],
                        in_=fo[:],
                    )

                # K/Q for unit (0,0) before attention starts
                k_group(0, 0)
                q_group(0, 0)
                q_group(0, 1)

                if dbg:
                    nc.sync.dma_start(out=d_kt[:], in_=kt_sb[0][:])
                    nc.sync.dma_start(out=d_qt[:], in_=qt_sb[0][:])

                def spread(items, nsteps=N_SC):
                    outl = [[] for _ in range(nsteps)]
                    for i, it in enumerate(items):
                        outl[(i * nsteps) // len(items)].append(it)
                    return outl

                def K(p, g):
                    return lambda: k_group(p, g)

                def Q(p, g):
                    return lambda: q_group(p, g)

                def V(sc, hp):
                    return lambda: v_group(sc, hp)

                def ST(qc, dmc):
                    return lambda: stage(qc, dmc)

                def FC(qc, dmc):
                    return lambda: full_chain(qc, dmc)

                def PF(p):
                    return [lambda p=p: prefetch_wk(p), lambda p=p: prefetch_wq(p)]

                # per-unit filler schedules (unit = 2*p + g, pair-major);
                # deadlines: v(sc,hp) by unit 2hp step sc+2; k(p,g) by unit
                # 2p step 4g; q(p,g01) by end of unit 2p-1; q(p,g23) by end
                # of unit 2p
                sched = {}
                # u0: v pair-0 (just in time), rest of K0, Q0 g1-half
                sched[0] = [[V(0, 0), V(1, 0), K(0, 1)]] + [
                    [V(sc + 1, 0)] for sc in range(1, N_SC - 1)
                ] + [[]]
                extras0 = [K(0, 2), K(0, 3), Q(0, 2), Q(0, 3)] + PF(1)
                for i, f in enumerate(extras0):
                    sched[0][3 + i].append(f)
                # u1: v pair-1 first half, K1
                sched[1] = spread(
                    [V(sc, 1) for sc in range(10)]
                    + [K(1, g) for g in range(4)]
                    + [Q(1, 0), Q(1, 1)]
                    + PF(2)
                )
                # u2: v pair-1 rest (deadlines step sc+2 inside this unit),
                # Q1 g1-half, K2
                s2 = [[] for _ in range(N_SC)]
                for sc in range(10, N_SC):
                    s2[sc - 9].append(V(sc, 1))
                for i, f in enumerate(
                    [K(2, 0), Q(1, 2), Q(1, 3), K(2, 1), K(2, 2), K(2, 3)]
                ):
                    s2[2 * i + 1].append(f)
                sched[2] = s2
                # u3: v pair-2, Q2 g0-half, K3 start
                sched[3] = spread(
                    PF(3)
                    + [V(sc, 2) for sc in range(10)]
                    + [Q(2, 0), Q(2, 1)]
                    + [K(3, 0)]
                )
                # u4: v pair-2 rest, Q2 g1-half, K3
                s4 = [[] for _ in range(N_SC)]
                for sc in range(10, N_SC):
                    s4[sc - 9].append(V(sc, 2))
                for i, f in enumerate([Q(2, 2), Q(2, 3), K(3, 1), K(3, 2)]):
                    s4[2 * i + 1].append(f)
                sched[4] = s4
                # u5: K3 rest, Q3, v pair-3 start
                sched[5] = spread(
                    [K(3, 3)]
                    + [V(sc, 3) for sc in range(10)]
                    + [Q(3, g) for g in range(4)]
                )
                # u6: v pair-3 rest, g1 stage chains
                s6 = [[] for _ in range(N_SC)]
                for sc in range(10, N_SC):
                    s6[sc - 9].append(V(sc, 3))
                for i, (qc, dmc) in enumerate(
                    [(qc, dmc) for qc in range(8, 14) for dmc in range(2)]
                ):
                    s6[i + 3].append(ST(qc, dmc))
                sched[6] = s6
                # u7: remaining g1 stages + full g0 chains
                sched[7] = spread(
                    [ST(qc, dmc) for qc in range(14, 16) for dmc in range(2)]
                    + [FC(qc, dmc) for qc in range(8) for dmc in range(2)]
                )

                # o accumulator slot -> AP. 16 slots (h,qc) packed into PSUM
                # banks as 7+7+2 (bank-straddle constraint).
                def o_slot(tiles, h, qc):
                    s = h * 8 + qc
                    if s < 7:
                        return tiles[0][:, s, :]
                    if s < 14:
                        return tiles[1][:, s - 7, :]
                    return tiles[2][:, s - 14, :]

                def emit_o(p, sc, h, o_ps, e_tiles):
                    e_sb = e_tiles.pop((sc, h))
                    for qc in range(8):
                        s = h * 8 + qc
                        nc.tensor.matmul(
                            o_slot(o_ps, h, qc),
                            e_sb[:, qc * 128 : (qc + 1) * 128],
                            v_all[:, sc, 2 * p + h, :],
                            start=(sc == 0 and s in (0, 7, 14)),
                            stop=(sc == N_SC - 1),
                            skip_group_check=True,
                        )

                # ---------------- attention (phase 2) ----------------
                for u in range(2 * NPC if 2 in phases else 0):
                    p, g = u // 2, u % 2
                    if dbg and u == 2:
                        nc.sync.dma_start(out=d_onorm[:], in_=qt_sb[0][:, 0:QW])
                    if dbg and u == 6:
                        nc.sync.dma_start(
                            out=d_vall[:],
                            in_=v_all[:].rearrange("p a b c -> p (a b c)"),
                        )
                    if u == 6:
                        # V work done; release its staging space and load W_O
                        ves.close()
                        nc.sync.dma_start(out=wo_sb[:], in_=w_o[:])
                    o_ps = [
                        opsum.tile([128, 7, 65], F32, tag="oA", name="oA"),
                        opsum.tile([128, 7, 65], F32, tag="oB", name="oB"),
                        opsum.tile([128, 2, 65], F32, tag="oC", name="oC"),
                    ]
                    e_tiles = {}
                    for sc in range(N_SC):
                        for h in range(2):
                            lo, hi = h * 64, h * 64 + 64
                            sp = spsum.tile([128, QW], F32, tag=f"sp{h}")
                            for qc2 in range(QW // 512):
                                nc.tensor.matmul(
                                    sp[:, qc2 * 512 : (qc2 + 1) * 512],
                                    kt_sb[p][lo:hi, sc * 128 : (sc + 1) * 128],
                                    qt_sb[p][
                                        lo:hi,
                                        g * QW + qc2 * 512 : g * QW + (qc2 + 1) * 512,
                                    ],
                                    start=True,
                                    stop=True,
                                    skip_group_check=True,
                                )
                            e_sb = epool.tile([128, QW], BF16, tag=f"e{h}")
                            nc.scalar.activation(
                                e_sb[:],
                                sp[:],
                                mybir.ActivationFunctionType.Exp,
                                scale=0.125,
                            )
                            e_tiles[(sc, h)] = e_sb
                        # value matmuls lag two s-chunks behind the scores
                        if sc > 1:
                            for h in range(2):
                                emit_o(p, sc - 2, h, o_ps, e_tiles)
                        for fill in sched[u][sc]:
                            fill()
                    for h in range(2):
                        emit_o(p, N_SC - 2, h, o_ps, e_tiles)
                    emit_o(p, N_SC - 1, 0, o_ps, e_tiles)
                    # drain PSUM -> SBUF (bf16) + f32 denominators; oA only
                    # holds h=0 slots so it can drain while h=1 value
                    # matmuls still run
                    o_sb = npool.tile([128, 16, 65], BF16, tag="osb")
                    den = npool.tile([128, 16], F32, tag="den")
                    nc.vector.tensor_copy(o_sb[:, 0:7, :], o_ps[0][:])
                    nc.vector.tensor_copy(den[:, 0:7, None], o_ps[0][:, :, 0:1])
                    emit_o(p, N_SC - 1, 1, o_ps, e_tiles)
                    nc.vector.tensor_copy(o_sb[:, 7:14, :], o_ps[1][:])
                    nc.vector.tensor_copy(o_sb[:, 14:16, :], o_ps[2][:])
                    nc.vector.tensor_copy(den[:, 7:14, None], o_ps[1][:, :, 0:1])
                    nc.vector.tensor_copy(den[:, 14:16, None], o_ps[2][:, :, 0:1])
                    rec = npool.tile([128, 16], F32, tag="rec")
                    nc.vector.reciprocal_approx_fast(rec[:], den[:])
                    # normalized flipped heads, qc-major, into the dead qt
                    # half: col = qc*128 + h*64 + v
                    o_norm = qt_sb[p][:, g * QW : (g + 1) * QW].rearrange(
                        "p (a b c) -> p b a c", a=8, b=2, c=64
                    )
                    nc.vector.tensor_mul(
                        o_norm,
                        o_sb[:, :, 1:65].rearrange("p (h q) c -> p h q c", h=2),
                        rec[:].rearrange("p (h q) -> p h q", h=2)[
                            :, :, :, None
                        ].broadcast_to([128, 2, 8, 64]),
                    )
                    # xbar transposes -> pair-stacked headsT, emitted only
                    # once the destination is dead: pairs 0-2 overwrite their
                    # kt tile after unit 2p+1's last scores; pair 3 goes to
                    # ktx (g0 after unit 6, g1 after unit 7)
                    if p < NPC - 1 and g == 1:
                        for g2 in range(2):
                            nc.sync.dma_start_transpose(
                                out=heads_dst(p, g2),
                                in_=qt_sb[p][:, g2 * QW : (g2 + 1) * QW],
                            )
                    elif p == NPC - 1:
                        nc.sync.dma_start_transpose(
                            out=heads_dst(p, g),
                            in_=qt_sb[p][:, g * QW : (g + 1) * QW],
                        )

                if dbg:
                    nc.sync.dma_start(out=d_heads[:], in_=kt_sb[0][:])

                # drain unused fillers (for phases subsets)
                if 2 not in phases:
                    ves.close()
                    nc.sync.dma_start(out=wo_sb[:], in_=w_o[:])
                    for u in range(2 * NPC):
                        for step in sched.get(u, []):
                            for fill in step:
                                fill()

            # ---------------- g1 out-projection tails ----------------
            # pair-3 matmul plus an identity(float32r) re-feed of the staged
            # pairs-0-2 partial into the same PSUM accumulation, then DMA
            # the finished chunk straight from PSUM.
            with (
                tc.tile_pool(name="psum_t", bufs=3, space="PSUM") as tpsum,
                tc.tile_pool(name="fout2", bufs=4) as fout2,
            ):
                if 3 in phases:
                    def tail2(qc, dmc):
                        tl = tpsum.tile([128, 512], F32, tag="tl")
                        nc.tensor.matmul(
                            tl[:],
                            heads_chunk(NPC - 1, qc),
                            wo_sb[:, NPC - 1, dmc * 512 : (dmc + 1) * 512],
                            start=True,
                            stop=False,
                        )
                        nc.tensor.matmul(
                            tl[:],
                            ident_bf[:],
                            stage_slot((qc - 8) * 2 + dmc),
                            start=False,
                            stop=True,
                        )
                        fo = fout2.tile([128, 512], F32, tag="fo")
                        if (qc + dmc) % 2 == 0:
                            nc.vector.tensor_copy(fo[:], tl[:])
                        else:
                            nc.scalar.copy(fo[:], tl[:])
                        nc.sync.dma_start(
                            out=out[
                                qc * 128 : (qc + 1) * 128,
                                dmc * 512 : (dmc + 1) * 512,
                            ],
                            in_=fo[:],
                        )

                    for s in range(N_SC):
                        tail2(8 + s // 2, s % 2)
    nc.compile()
    return nc


_NC_CACHE = {}


def _get_nc():
    if "nc" not in _NC_CACHE:
        _NC_CACHE["nc"] = build()
    return _NC_CACHE["nc"]


def _prep_w3p(w):
    # [H, DM, D] -> pair-major [8 pairs, mi=128, mo=8, 128], bf16
    w3 = w.transpose(1, 0, 2).reshape(N_MO, 128, H * D).transpose(1, 0, 2)
    return np.ascontiguousarray(
        w3.reshape(128, N_MO, H // 2, 128).transpose(2, 0, 1, 3)
    ).astype(ml_dtypes.bfloat16)


def _prep_wv(w):
    # [H, DM, D] -> [mi=128, mo=8, (h d)=1024], bf16
    return np.ascontiguousarray(
        w.transpose(1, 0, 2).reshape(N_MO, 128, H * D).transpose(1, 0, 2)
    ).astype(ml_dtypes.bfloat16)


def _prep_wo(w):
    # [H*D=1024, DM] -> [mi=128, chunk=8, DM], bf16
    return np.ascontiguousarray(w.reshape(8, 128, DM).transpose(1, 0, 2)).astype(
        ml_dtypes.bfloat16
    )


def _prep_xt(x):
    # [S, DM] -> transposed [128(dm within mo), mo=8, S], bf16
    return np.ascontiguousarray(
        x.T.reshape(N_MO, 128, x.shape[0]).transpose(1, 0, 2)
    ).astype(ml_dtypes.bfloat16)


def _prep_vt(x):
    # [S, DM] -> [128(dm within mo), sc=16, mo=8, 128(s within chunk)]
    return np.ascontiguousarray(
        x.reshape(N_SC, 128, N_MO, 128).transpose(3, 0, 2, 1)
    ).astype(ml_dtypes.bfloat16)


def kernel(Q, K, V, W_Q, W_K, W_V, W_O, _trace=False):
    Q = np.asarray(Q, dtype=np.float32)
    K = np.asarray(K, dtype=np.float32)
    V = np.asarray(V, dtype=np.float32)
    wq8 = _prep_w3p(np.asarray(W_Q, dtype=np.float32))  # [8 pairs, ...]
    wk8 = _prep_w3p(np.asarray(W_K, dtype=np.float32))
    wv8 = _prep_wv(np.asarray(W_V, dtype=np.float32))  # [128, 8, 1024]
    wo8 = _prep_wo(np.asarray(W_O, dtype=np.float32))  # [128, 8, DM]

    qt_b = [_prep_xt(Q[b]) for b in range(B)]
    kt_b = [_prep_xt(K[b]) for b in range(B)]
    vt_b = [_prep_vt(V[b]) for b in range(B)]
    wq_h = [np.ascontiguousarray(wq8[hh * NPC : (hh + 1) * NPC]) for hh in range(2)]
    wk_h = [np.ascontiguousarray(wk8[hh * NPC : (hh + 1) * NPC]) for hh in range(2)]
    wv_h = [
        np.ascontiguousarray(wv8[:, :, hh * 512 : (hh + 1) * 512]) for hh in range(2)
    ]
    wo_h = [
        np.ascontiguousarray(wo8[:, hh * NPC : (hh + 1) * NPC, :]) for hh in range(2)
    ]

    in_maps = []
    for c in range(N_CORES):
        b, hh = c // 2, c % 2
        in_maps.append(
            {
                "QT": qt_b[b],
                "KT": kt_b[b],
                "VTs": vt_b[b],
                "WQP": wq_h[hh],
                "WKP": wk_h[hh],
                "WV3": wv_h[hh],
                "WO3": wo_h[hh],
            }
        )

    nc = _get_nc()
    res = run_bass_kernel_spmd(nc, in_maps, list(range(N_CORES)), trace=_trace)
    out = np.empty((B, S, DM), dtype=np.float32)
    for b in range(B):
        out[b] = res.results[2 * b]["out"] + res.results[2 * b + 1]["out"]
    if _trace:
        kernel._last_results = res
    return out


# revision 14
# speedup vs baseline: 1.1452x; 1.0166x over previous
"""Multi-head attention (B=4, S=2048, H=16, d_model=1024, d_k=d_v=64) on 8
Trainium2 NeuronCores.

Sharding (v2): 8 cores = 4 batches x 2 head-halves (tensor-parallel over
heads, per the W_Q/W_K/W_V head-split + W_O row-split scheme). Each core
computes 8 heads (4 pairs) over the FULL query range S=2048 for its batch,
projects K/V only for its own heads (no duplicated projection work), runs
its partial output projection against its W_O row block, and the host sums
the two partial outputs per batch (the all-reduce).

Host prep: Q/K/V transposed to [d_model, seq] (V blocked by s-chunk) and
cast to bf16, as are all weights; W_Q/W_K pair-major; W_V/W_O sliced per
head-half.

Per-core pipeline: 8 attention units (pair p, query-half g), pair-major
order. Per unit, the baseline-style software pipeline: scoresT chunk =
kt.T @ qt into PSUM, e = exp(s/8) on ACT (the pacing engine), flipped
value matmul o[q, 65] = e-chunk.T @ [1|v] accumulated over s-chunks with
column 0 the softmax denominator. K/Q/V projection groups and the
output-projection partials run as PE filler inside the ACT-paced loop.

Normalized heads are written qc-major into the unit's dead qt half, then a
single DMA xbar transpose per unit produces pair-stacked headsT in the dead
ktx staging area (no PE transposes). Output projection: partial chains over
pairs 0-2 are staged to SBUF f32 (dead qtx space) as filler; pair-3 tail
matmul + DVE/GPSIMD add completes each chunk (g0 chunks inside unit 7,
g1 chunks in the drain tail).

PSUM note: start_tensor_calc marks the whole 2KB bank pending-zero, so
interleaved per-slot accumulation groups sharing a bank must issue exactly
one start (first slot); the other slots' first writes land on pending-zero
bytes, which the hardware treats as overwrite.
"""

import contextlib
import os
import sys

for _p in ("/opt/trn_rl_repo", "/root/.axon_site/_ro/trn_rl_repo"):
    if os.path.isdir(_p) and _p not in sys.path:
        sys.path.insert(0, _p)

import numpy as np
import ml_dtypes

import concourse.bass as bass  # noqa: F401
import concourse.tile as tile
from concourse import bacc, mybir
from concourse.bass_utils import run_bass_kernel_spmd
from concourse.masks import make_identity

F32 = mybir.dt.float32
F32R = mybir.dt.float32r
BF16 = mybir.dt.bfloat16

B, S, DM = 4, 2048, 1024
H, D = 16, 64
N_CORES = 8
NPC = 4  # head pairs per core (8 heads)
N_SC = S // 128  # kv 128-chunks
N_MO = DM // 128  # model-dim 128-chunks
N_G = 2  # query halves per core
QW = 1024  # query width per attention unit


def build(n_cores=N_CORES, phases=(1, 2, 3), dbg=False):
    nc = bacc.Bacc("TRN2", target_bir_lowering=False, debug=False, num_devices=n_cores)

    # host-transposed activations, bf16 (full batch; core's own head slice
    # of the weights)
    qt_d = nc.dram_tensor("QT", [128, N_MO, S], BF16, kind="ExternalInput").ap()
    kt_d = nc.dram_tensor("KT", [128, N_MO, S], BF16, kind="ExternalInput").ap()
    vt_d = nc.dram_tensor(
        "VTs", [128, N_SC, N_MO, 128], BF16, kind="ExternalInput"
    ).ap()
    # pair-major: [pair, mi=128, mo=8, 128]
    w_q = nc.dram_tensor("WQP", [NPC, 128, N_MO, 128], BF16, kind="ExternalInput").ap()
    w_k = nc.dram_tensor("WKP", [NPC, 128, N_MO, 128], BF16, kind="ExternalInput").ap()
    w_v = nc.dram_tensor("WV3", [128, N_MO, 8 * D], BF16, kind="ExternalInput").ap()
    # [mi=128, pair=4, dm=1024]
    w_o = nc.dram_tensor("WO3", [128, NPC, DM], BF16, kind="ExternalInput").ap()
    out = nc.dram_tensor("out", [S, DM], BF16, kind="ExternalOutput").ap()
    if dbg:
        d_kt = nc.dram_tensor("d_kt", [128, S], BF16, kind="ExternalOutput").ap()
        d_qt = nc.dram_tensor("d_qt", [128, S], BF16, kind="ExternalOutput").ap()
        d_vall = nc.dram_tensor(
            "d_vall", [128, N_SC * 8 * 65], BF16, kind="ExternalOutput"
        ).ap()
        d_onorm = nc.dram_tensor("d_onorm", [128, QW], BF16, kind="ExternalOutput").ap()
        d_heads = nc.dram_tensor("d_heads", [128, S], BF16, kind="ExternalOutput").ap()

    with tile.TileContext(nc) as tc:
        with (
            tc.tile_pool(name="pers", bufs=1) as pers,
            tc.tile_pool(name="wkq", bufs=2) as wkq,
            tc.tile_pool(name="wop", bufs=1) as wop,
        ):
            wo_sb = wop.tile([128, NPC, DM], BF16, tag="wo", name="wo")
            ident_f32 = wop.tile([128, 128], F32, tag="identf", name="ident_f32")
            make_identity(nc, ident_f32[:])
            ident_bf = wop.tile([128, 128], BF16, tag="ident", name="ident_bf")
            nc.vector.tensor_copy(ident_bf[:], ident_f32[:])
            # v resident: per s-chunk block of 8 head-slots [1|v] (65 wide)
            v_all = pers.tile([128, N_SC, 8, 65], BF16, tag="v_all")
            nc.vector.memset(v_all[:, :, :, 0:1], 1.0)
            # kt[p]: pair-stacked [2*64, S]; qt[p]: [128, S], whose g-halves
            # are later reused for normalized flipped heads (qc-major)
            kt_sb = [
                pers.tile([128, S], BF16, tag=f"kt{p}", name=f"kt{p}")
                for p in range(NPC)
            ]
            qt_sb = [
                pers.tile([128, S], BF16, tag=f"qt{p}", name=f"qt{p}")
                for p in range(NPC)
            ]
            # resident staged transposed inputs for projections; ktx rows
            # double as headsT homes (mo-slot p holds pair p's headsT) once
            # the K projections have consumed them
            ktx = pers.tile([128, N_MO, S], BF16, tag="ktx")
            qtx = pers.tile([128, N_MO * S], BF16, tag="qtx")

            def heads_home(p):
                # pairs 0-2: own kt tile (dead after unit 2p+1's scores);
                # pair 3: ktx slot 3 (dead once the last K projection group
                # has run, i.e. from unit 6 on)
                return kt_sb[p] if p < NPC - 1 else ktx[:, NPC - 1, :]

            def heads_dst(p, g):
                # [128, 8, 128] chunked-transpose target
                return heads_home(p)[:, g * QW : (g + 1) * QW].rearrange(
                    "p (c q) -> p c q", q=128
                )

            def heads_chunk(p, qc):
                # out-projection stationary: [128 hv, 128 q] for global qc
                return heads_home(p)[:, qc * 128 : (qc + 1) * 128]

            def stage_slot(i):
                # 16 bf16 staging slots in dead qtx space (bf16 so the tail
                # identity-matmul re-feed stays a plain bf16 matmul)
                return qtx[:, i * 512 : (i + 1) * 512]

            with (
                tc.tile_pool(name="psum_sp", bufs=1, space="PSUM") as spsum,
                tc.tile_pool(name="psum_o", bufs=1, space="PSUM") as opsum,
                tc.tile_pool(name="psum_pj", bufs=1, space="PSUM") as pjsum,
                tc.tile_pool(name="epool", bufs=4) as epool,
                tc.tile_pool(name="npool", bufs=1) as npool,
                tc.tile_pool(name="fout", bufs=2) as fout,
            ):
                ves = contextlib.ExitStack()
                vpool = ves.enter_context(tc.tile_pool(name="vpool", bufs=1))
                vtx = vpool.tile([128, N_SC, N_MO, 128], BF16, tag="vtx")
                wv_sb = vpool.tile([128, N_MO, 8 * D], BF16, tag="wv")

                cur_wk, cur_wq = {}, {}

                def prefetch_wk(p):
                    wkt = wkq.tile([128, N_MO, 128], BF16, tag="wk", name="wk")
                    cur_wk[p] = wkt
                    nc.sync.dma_start(out=wkt[:], in_=w_k[p])

                def prefetch_wq(p):
                    wqt = wkq.tile([128, N_MO, 128], BF16, tag="wq", name="wq")
                    cur_wq[p] = wqt
                    nc.sync.dma_start(out=wqt[:], in_=w_q[p])

                def dma_ktx(g):
                    nc.sync.dma_start(
                        out=ktx[:, :, g * 512 : (g + 1) * 512],
                        in_=kt_d[:, :, g * 512 : (g + 1) * 512],
                    )

                def dma_qtx(g):
                    nc.sync.dma_start(
                        out=qtx[:].rearrange("p (m s) -> p m s", s=S)[
                            :, :, g * 512 : (g + 1) * 512
                        ],
                        in_=qt_d[:, :, g * 512 : (g + 1) * 512],
                    )

                def dma_wv(first):
                    if first:
                        nc.sync.dma_start(
                            out=wv_sb[:, :, 0:128], in_=w_v[:, :, 0:128]
                        )
                    else:
                        nc.sync.dma_start(
                            out=wv_sb[:, :, 128:512], in_=w_v[:, :, 128:512]
                        )

                def dma_vt(blk):
                    nc.sync.dma_start(
                        out=vtx[:, 4 * blk : 4 * blk + 4, :, :],
                        in_=vt_d[:, 4 * blk : 4 * blk + 4, :, :],
                    )

                # startup-critical DMA order: pair-0 K/Q weights + first K/Q
                # columns, then V / remaining K/Q interleaved
                prefetch_wk(0)
                prefetch_wq(0)
                dma_ktx(0)
                dma_qtx(0)
                dma_qtx(1)
                dma_wv(True)
                dma_vt(0)
                prefetch_wk(1)
                prefetch_wq(1)
                dma_ktx(1)
                dma_vt(1)
                dma_ktx(2)
                dma_vt(2)
                dma_ktx(3)
                dma_qtx(2)
                dma_vt(3)
                dma_qtx(3)
                dma_wv(False)

                # ---------- projection "filler" groups ----------
                def k_group(p, g):
                    pj = pjsum.tile([128, 512], F32, tag="pj")
                    for mo in range(N_MO):
                        nc.tensor.matmul(
                            pj[:],
                            cur_wk[p][:, mo, :],
                            ktx[:, mo, g * 512 : (g + 1) * 512],
                            start=(mo == 0),
                            stop=(mo == N_MO - 1),
                        )
                    nc.vector.tensor_copy(kt_sb[p][:, g * 512 : (g + 1) * 512], pj[:])

                def q_group(p, g):
                    pj = pjsum.tile([128, 512], F32, tag="pj")
                    for mo in range(N_MO):
                        nc.tensor.matmul(
                            pj[:],
                            cur_wq[p][:, mo, :],
                            qtx[:, mo * S + g * 512 : mo * S + (g + 1) * 512],
                            start=(mo == 0),
                            stop=(mo == N_MO - 1),
                        )
                    nc.vector.tensor_copy(qt_sb[p][:, g * 512 : (g + 1) * 512], pj[:])

                def v_group(sc, hp):
                    # one pair's two head-slots (128 wv cols), chunk sc
                    pj = pjsum.tile([128, 512], F32, tag="pj")
                    for mo in range(N_MO):
                        nc.tensor.matmul(
                            pj[:, 0:128],
                            vtx[:, sc, mo, :],
                            wv_sb[:, mo, hp * 128 : (hp + 1) * 128],
                            start=(mo == 0),
                            stop=(mo == N_MO - 1),
                        )
                    nc.vector.tensor_copy(
                        v_all[:, sc, 2 * hp : 2 * hp + 2, 1:65],
                        pj[:, 0:128].rearrange("p (h w) -> p h w", h=2),
                    )

                # out-projection:
                #  - g1 chunks: partial chains over pairs 0-2 staged to SBUF
                #    f32 (dead qtx space) inside units 6-7; the drain tail
                #    re-feeds each staged partial into PSUM with an identity
                #    matmul (float32r moving operand: full rate, ~tf32
                #    rounding) on top of the pair-3 matmul, then DMAs the
                #    PSUM result out directly.
                #  - g0 chunks: full 4-pair chains inside unit 7.
                def stage(qc, dmc):
                    pj = pjsum.tile([128, 512], F32, tag="pj", name="st")
                    for p2 in range(NPC - 1):
                        nc.tensor.matmul(
                            pj[:],
                            heads_chunk(p2, qc),
                            wo_sb[:, p2, dmc * 512 : (dmc + 1) * 512],
                            start=(p2 == 0),
                            stop=(p2 == NPC - 2),
                        )
                    nc.vector.tensor_copy(stage_slot((qc - 8) * 2 + dmc), pj[:])

                def full_chain(qc):
                    # both dm-halves of one q-chunk -> one bf16 store
                    fo = fout.tile([128, 1024], BF16, tag="fo")
                    for dmc in range(2):
                        pj = pjsum.tile([128, 512], F32, tag="pj", name="fc")
                        for p2 in range(NPC):
                            nc.tensor.matmul(
                                pj[:],
                                heads_chunk(p2, qc),
                                wo_sb[:, p2, dmc * 512 : (dmc + 1) * 512],
                                start=(p2 == 0),
                                stop=(p2 == NPC - 1),
                            )
                        nc.vector.tensor_copy(
                            fo[:, dmc * 512 : (dmc + 1) * 512], pj[:]
                        )
                    nc.sync.dma_start(out=out[qc * 128 : (qc + 1) * 128, :], in_=fo[:])

                # K/Q for unit (0,0) before attention starts
                k_group(0, 0)
                q_group(0, 0)
                q_group(0, 1)

                if dbg:
                    nc.sync.dma_start(out=d_kt[:], in_=kt_sb[0][:])
                    nc.sync.dma_start(out=d_qt[:], in_=qt_sb[0][:])

                def spread(items, nsteps=N_SC):
                    outl = [[] for _ in range(nsteps)]
                    for i, it in enumerate(items):
                        outl[(i * nsteps) // len(items)].append(it)
                    return outl

                def K(p, g):
                    return lambda: k_group(p, g)

                def Q(p, g):
                    return lambda: q_group(p, g)

                def V(sc, hp):
                    return lambda: v_group(sc, hp)

                def ST(qc, dmc):
                    return lambda: stage(qc, dmc)

                def FC(qc):
                    return lambda: full_chain(qc)

                def PF(p):
                    return [lambda p=p: prefetch_wk(p), lambda p=p: prefetch_wq(p)]

                # per-unit filler schedules (unit = 2*p + g, pair-major);
                # deadlines: v(sc,hp) by unit 2hp step sc+2; k(p,g) by unit
                # 2p step 4g; q(p,g01) by end of unit 2p-1; q(p,g23) by end
                # of unit 2p
                sched = {}
                # u0: v pair-0 and K0 interleaved in DMA-arrival order so
                # a stalled group never blocks the pj bank for a ready one;
                # Q0 g1-half last (its staging DMA arrives late)
                sched[0] = [[] for _ in range(N_SC)]
                for step, fills in {
                    0: [V(0, 0), V(1, 0)],
                    1: [V(2, 0)],
                    2: [V(3, 0)],
                    3: [K(0, 1)],
                    4: [V(4, 0)],
                    5: [V(5, 0)],
                    6: [V(6, 0)],
                    7: [V(7, 0), K(0, 2)],
                    8: [V(8, 0)],
                    9: [V(9, 0)],
                    10: [V(10, 0), K(0, 3)],
                    11: [V(11, 0)],
                    12: [V(12, 0), Q(0, 2)],
                    13: [V(13, 0), Q(0, 3)],
                    14: [V(14, 0)],
                    15: [V(15, 0)],
                }.items():
                    sched[0][step] = fills
                # u1: v pair-0 leftovers + v pair-1 first half, K1
                sched[1] = spread(
                    [V(sc, 1) for sc in range(10)]
                    + [K(1, g) for g in range(4)]
                    + [Q(1, 0), Q(1, 1)]
                    + PF(2)
                )
                # u2: v pair-1 rest (deadlines step sc+2 inside this unit),
                # Q1 g1-half, K2
                s2 = [[] for _ in range(N_SC)]
                for sc in range(10, N_SC):
                    s2[sc - 9].append(V(sc, 1))
                for i, f in enumerate(
                    [K(2, 0), Q(1, 2), Q(1, 3), K(2, 1), K(2, 2), K(2, 3)]
                ):
                    s2[2 * i + 1].append(f)
                sched[2] = s2
                # u3: v pair-2, Q2 g0-half, K3 start
                sched[3] = spread(
                    PF(3)
                    + [V(sc, 2) for sc in range(10)]
                    + [Q(2, 0), Q(2, 1)]
                    + [K(3, 0)]
                )
                # u4: v pair-2 rest, Q2 g1-half, K3
                s4 = [[] for _ in range(N_SC)]
                for sc in range(10, N_SC):
                    s4[sc - 9].append(V(sc, 2))
                for i, f in enumerate([Q(2, 2), Q(2, 3), K(3, 1), K(3, 2)]):
                    s4[2 * i + 1].append(f)
                sched[4] = s4
                # u5: K3 rest, Q3, v pair-3 start
                sched[5] = spread(
                    [K(3, 3)]
                    + [V(sc, 3) for sc in range(10)]
                    + [Q(3, g) for g in range(4)]
                )
                # u6: v pair-3 rest, g1 stage chains
                s6 = [[] for _ in range(N_SC)]
                for sc in range(10, N_SC):
                    s6[sc - 9].append(V(sc, 3))
                for i, (qc, dmc) in enumerate(
                    [(qc, dmc) for qc in range(8, 14) for dmc in range(2)]
                ):
                    s6[i + 3].append(ST(qc, dmc))
                sched[6] = s6
                # u7: remaining g1 stages + full g0 chains
                sched[7] = spread(
                    [ST(qc, dmc) for qc in range(14, 16) for dmc in range(2)]
                    + [FC(qc) for qc in range(8)]
                )

                # o accumulator slot -> AP. 16 slots (h,qc) packed into PSUM
                # banks as 7+7+2 (bank-straddle constraint).
                def o_slot(tiles, h, qc):
                    s = h * 8 + qc
                    if s < 7:
                        return tiles[0][:, s, :]
                    if s < 14:
                        return tiles[1][:, s - 7, :]
                    return tiles[2][:, s - 14, :]

                def emit_o(p, sc, h, o_ps, e_tiles):
                    e_sb = e_tiles.pop((sc, h))
                    for qc in range(8):
                        s = h * 8 + qc
                        nc.tensor.matmul(
                            o_slot(o_ps, h, qc),
                            e_sb[:, qc * 128 : (qc + 1) * 128],
                            v_all[:, sc, 2 * p + h, :],
                            start=(sc == 0 and s in (0, 7, 14)),
                            stop=(sc == N_SC - 1),
                            skip_group_check=True,
                        )

                # ---------------- attention (phase 2) ----------------
                for u in range(2 * NPC if 2 in phases else 0):
                    p, g = u // 2, u % 2
                    if dbg and u == 2:
                        nc.sync.dma_start(out=d_onorm[:], in_=qt_sb[0][:, 0:QW])
                    if dbg and u == 6:
                        nc.sync.dma_start(
                            out=d_vall[:],
                            in_=v_all[:].rearrange("p a b c -> p (a b c)"),
                        )
                    if u == 6:
                        # V work done; release its staging space and load W_O
                        ves.close()
                        nc.sync.dma_start(out=wo_sb[:], in_=w_o[:])
                    o_ps = [
                        opsum.tile([128, 7, 65], F32, tag="oA", name="oA"),
                        opsum.tile([128, 7, 65], F32, tag="oB", name="oB"),
                        opsum.tile([128, 2, 65], F32, tag="oC", name="oC"),
                    ]
                    e_tiles = {}
                    for sc in range(N_SC):
                        for h in range(2):
                            lo, hi = h * 64, h * 64 + 64
                            sp = spsum.tile([128, QW], F32, tag=f"sp{h}")
                            for qc2 in range(QW // 512):
                                nc.tensor.matmul(
                                    sp[:, qc2 * 512 : (qc2 + 1) * 512],
                                    kt_sb[p][lo:hi, sc * 128 : (sc + 1) * 128],
                                    qt_sb[p][
                                        lo:hi,
                                        g * QW + qc2 * 512 : g * QW + (qc2 + 1) * 512,
                                    ],
                                    start=True,
                                    stop=True,
                                    skip_group_check=True,
                                )
                            e_sb = epool.tile([128, QW], BF16, tag=f"e{h}")
                            nc.scalar.activation(
                                e_sb[:],
                                sp[:],
                                mybir.ActivationFunctionType.Exp,
                                scale=0.125,
                            )
                            e_tiles[(sc, h)] = e_sb
                        # value matmuls lag two s-chunks behind the scores
                        if sc > 1:
                            for h in range(2):
                                emit_o(p, sc - 2, h, o_ps, e_tiles)
                        for fill in sched[u][sc]:
                            fill()
                    for h in range(2):
                        emit_o(p, N_SC - 2, h, o_ps, e_tiles)
                    emit_o(p, N_SC - 1, 0, o_ps, e_tiles)
                    # drain PSUM -> SBUF (bf16) + f32 denominators; oA only
                    # holds h=0 slots so it can drain while h=1 value
                    # matmuls still run
                    o_sb = npool.tile([128, 16, 65], BF16, tag="osb")
                    den = npool.tile([128, 16], F32, tag="den")
                    nc.vector.tensor_copy(o_sb[:, 0:7, :], o_ps[0][:])
                    nc.vector.tensor_copy(den[:, 0:7, None], o_ps[0][:, :, 0:1])
                    emit_o(p, N_SC - 1, 1, o_ps, e_tiles)
                    nc.vector.tensor_copy(o_sb[:, 7:14, :], o_ps[1][:])
                    nc.vector.tensor_copy(o_sb[:, 14:16, :], o_ps[2][:])
                    nc.vector.tensor_copy(den[:, 7:14, None], o_ps[1][:, :, 0:1])
                    nc.vector.tensor_copy(den[:, 14:16, None], o_ps[2][:, :, 0:1])
                    rec = npool.tile([128, 16], F32, tag="rec")
                    nc.vector.reciprocal_approx_fast(rec[:], den[:])
                    # normalized flipped heads, qc-major, into the dead qt
                    # half: col = qc*128 + h*64 + v
                    o_norm = qt_sb[p][:, g * QW : (g + 1) * QW].rearrange(
                        "p (a b c) -> p b a c", a=8, b=2, c=64
                    )
                    nc.vector.tensor_mul(
                        o_norm,
                        o_sb[:, :, 1:65].rearrange("p (h q) c -> p h q c", h=2),
                        rec[:].rearrange("p (h q) -> p h q", h=2)[
                            :, :, :, None
                        ].broadcast_to([128, 2, 8, 64]),
                    )
                    # xbar transposes -> pair-stacked headsT, emitted only
                    # once the destination is dead: pairs 0-2 overwrite their
                    # kt tile after unit 2p+1's last scores; pair 3 goes to
                    # ktx (g0 after unit 6, g1 after unit 7)
                    if p < NPC - 1 and g == 1:
                        for g2 in range(2):
                            nc.sync.dma_start_transpose(
                                out=heads_dst(p, g2),
                                in_=qt_sb[p][:, g2 * QW : (g2 + 1) * QW],
                            )
                    elif p == NPC - 1:
                        nc.sync.dma_start_transpose(
                            out=heads_dst(p, g),
                            in_=qt_sb[p][:, g * QW : (g + 1) * QW],
                        )

                if dbg:
                    nc.sync.dma_start(out=d_heads[:], in_=kt_sb[0][:])

                # drain unused fillers (for phases subsets)
                if 2 not in phases:
                    ves.close()
                    nc.sync.dma_start(out=wo_sb[:], in_=w_o[:])
                    for u in range(2 * NPC):
                        for step in sched.get(u, []):
                            for fill in step:
                                fill()

            # ---------------- g1 out-projection tails ----------------
            # pair-3 matmul plus an identity(float32r) re-feed of the staged
            # pairs-0-2 partial into the same PSUM accumulation, then DMA
            # the finished chunk straight from PSUM.
            with (
                tc.tile_pool(name="psum_t", bufs=3, space="PSUM") as tpsum,
                tc.tile_pool(name="fout2", bufs=4) as fout2,
            ):
                if 3 in phases:
                    def tail2(qc):
                        fo = fout2.tile([128, 1024], BF16, tag="fo")
                        for dmc in range(2):
                            tl = tpsum.tile([128, 512], F32, tag="tl")
                            nc.tensor.matmul(
                                tl[:],
                                heads_chunk(NPC - 1, qc),
                                wo_sb[:, NPC - 1, dmc * 512 : (dmc + 1) * 512],
                                start=True,
                                stop=False,
                            )
                            nc.tensor.matmul(
                                tl[:],
                                ident_bf[:],
                                stage_slot((qc - 8) * 2 + dmc),
                                start=False,
                                stop=True,
                            )
                            if dmc == 0:
                                nc.scalar.copy(fo[:, 0:512], tl[:])
                            else:
                                nc.vector.tensor_copy(fo[:, 512:1024], tl[:])
                        nc.sync.dma_start(
                            out=out[qc * 128 : (qc + 1) * 128, :], in_=fo[:]
                        )

                    for qc in range(8, 16):
                        tail2(qc)
    nc.compile()
    return nc


_NC_CACHE = {}


def _get_nc():
    if "nc" not in _NC_CACHE:
        _NC_CACHE["nc"] = build()
    return _NC_CACHE["nc"]


def _prep_w3p(w):
    # [H, DM, D] -> pair-major [8 pairs, mi=128, mo=8, 128], bf16
    w3 = w.transpose(1, 0, 2).reshape(N_MO, 128, H * D).transpose(1, 0, 2)
    return np.ascontiguousarray(
        w3.reshape(128, N_MO, H // 2, 128).transpose(2, 0, 1, 3)
    ).astype(ml_dtypes.bfloat16)


def _prep_wv(w):
    # [H, DM, D] -> [mi=128, mo=8, (h d)=1024], bf16
    return np.ascontiguousarray(
        w.transpose(1, 0, 2).reshape(N_MO, 128, H * D).transpose(1, 0, 2)
    ).astype(ml_dtypes.bfloat16)


def _prep_wo(w):
    # [H*D=1024, DM] -> [mi=128, chunk=8, DM], bf16
    return np.ascontiguousarray(w.reshape(8, 128, DM).transpose(1, 0, 2)).astype(
        ml_dtypes.bfloat16
    )


def _prep_xt(x):
    # [S, DM] -> transposed [128(dm within mo), mo=8, S], bf16
    return np.ascontiguousarray(
        x.T.reshape(N_MO, 128, x.shape[0]).transpose(1, 0, 2)
    ).astype(ml_dtypes.bfloat16)


def _prep_vt(x):
    # [S, DM] -> [128(dm within mo), sc=16, mo=8, 128(s within chunk)]
    return np.ascontiguousarray(
        x.reshape(N_SC, 128, N_MO, 128).transpose(3, 0, 2, 1)
    ).astype(ml_dtypes.bfloat16)


def kernel(Q, K, V, W_Q, W_K, W_V, W_O, _trace=False):
    Q = np.asarray(Q, dtype=np.float32)
    K = np.asarray(K, dtype=np.float32)
    V = np.asarray(V, dtype=np.float32)
    wq8 = _prep_w3p(np.asarray(W_Q, dtype=np.float32))  # [8 pairs, ...]
    wk8 = _prep_w3p(np.asarray(W_K, dtype=np.float32))
    wv8 = _prep_wv(np.asarray(W_V, dtype=np.float32))  # [128, 8, 1024]
    wo8 = _prep_wo(np.asarray(W_O, dtype=np.float32))  # [128, 8, DM]

    qt_b = [_prep_xt(Q[b]) for b in range(B)]
    kt_b = [_prep_xt(K[b]) for b in range(B)]
    vt_b = [_prep_vt(V[b]) for b in range(B)]
    wq_h = [np.ascontiguousarray(wq8[hh * NPC : (hh + 1) * NPC]) for hh in range(2)]
    wk_h = [np.ascontiguousarray(wk8[hh * NPC : (hh + 1) * NPC]) for hh in range(2)]
    wv_h = [
        np.ascontiguousarray(wv8[:, :, hh * 512 : (hh + 1) * 512]) for hh in range(2)
    ]
    wo_h = [
        np.ascontiguousarray(wo8[:, hh * NPC : (hh + 1) * NPC, :]) for hh in range(2)
    ]

    in_maps = []
    for c in range(N_CORES):
        b, hh = c // 2, c % 2
        in_maps.append(
            {
                "QT": qt_b[b],
                "KT": kt_b[b],
                "VTs": vt_b[b],
                "WQP": wq_h[hh],
                "WKP": wk_h[hh],
                "WV3": wv_h[hh],
                "WO3": wo_h[hh],
            }
        )

    nc = _get_nc()
    res = run_bass_kernel_spmd(nc, in_maps, list(range(N_CORES)), trace=_trace)
    out = np.empty((B, S, DM), dtype=np.float32)
    for b in range(B):
        out[b] = np.asarray(res.results[2 * b]["out"], dtype=np.float32) + np.asarray(
            res.results[2 * b + 1]["out"], dtype=np.float32
        )
    if _trace:
        kernel._last_results = res
    return out
